# revision 15
# baseline (speedup 1.0000x reference)
"""Trainium2 Bass kernel for a minimal transformer block (B=2, T=2048, C=1024,
H=16, Dh=64, F=4096), sharded over 8 NeuronCores.

Sharding: data-parallel over batch (2 groups of 4 cores) x sequence-parallel
over tokens within each batch (512 query tokens per core). Each core
redundantly computes K/V for its batch's full sequence, which removes all
cross-core communication: the host only concatenates the per-core output
slices. Per-core token identity is established by rolling the sequence so the
core's own 512 tokens come first (softmax attention without a mask is
invariant to key/value ordering).

Everything on-chip is feature-major ([features, tokens]); the host transposes
inputs/outputs and pre-transposes/casts weights to bf16.
"""

import sys

if "/opt/trn_rl_repo" not in sys.path:
    sys.path.insert(0, "/opt/trn_rl_repo")

import numpy as np

D_MODEL = 1024
N_HEAD = 16
HEAD_DIM = 64
D_FF = 4096
B = 2
T = 2048
N_CORES = 8
GROUPS = 4          # cores per batch
TQ = T // GROUPS    # own query tokens per core = 512
P = 128
NCC = D_MODEL // P  # 8 C-chunks
NTC = T // 512      # 4 T-chunks of 512
NKC = T // P        # 16 k-chunks of 128
NFC = D_FF // P     # 32 f-chunks of 128

# bias-table column layout ([128, 64] f32)
QB, KB, OB, B1, B2 = 0, 8, 16, 24, 56

_cache = {}


def _build():
    import concourse.bass as bass
    import concourse.tile as tile
    from concourse import bacc, mybir

    f32 = mybir.dt.float32
    bf16 = mybir.dt.bfloat16
    AF = mybir.ActivationFunctionType
    OP = mybir.AluOpType

    nc = bacc.Bacc("TRN2", target_bir_lowering=False, debug=False,
                   num_devices=N_CORES)

    x_d = nc.dram_tensor("x_fm", [D_MODEL, T], bf16, kind="ExternalInput").ap()
    xo_d = nc.dram_tensor("x_own", [D_MODEL, TQ], f32,
                          kind="ExternalInput").ap()
    qkvw_d = nc.dram_tensor("qkv_wT", [D_MODEL, 3 * D_MODEL], bf16,
                            kind="ExternalInput").ap()
    wbar_d = nc.dram_tensor("wbar", [1, 3 * D_MODEL], bf16,
                            kind="ExternalInput").ap()
    wbar1_d = nc.dram_tensor("wbar1", [1, D_FF], bf16,
                             kind="ExternalInput").ap()
    ow_d = nc.dram_tensor("o_wT", [D_MODEL, D_MODEL], bf16,
                          kind="ExternalInput").ap()
    w1_d = nc.dram_tensor("w1T", [D_MODEL, D_FF], bf16,
                          kind="ExternalInput").ap()
    w2_d = nc.dram_tensor("w2T", [D_FF, D_MODEL], bf16,
                          kind="ExternalInput").ap()
    bias_d = nc.dram_tensor("biases", [P, 64], f32, kind="ExternalInput").ap()
    out_d = nc.dram_tensor("out", [D_MODEL, TQ], f32,
                           kind="ExternalOutput").ap()

    with tile.TileContext(nc) as tc:
        _body(tc, bass, mybir, f32, bf16, AF, OP, x_d, xo_d, qkvw_d, wbar_d,
              wbar1_d, ow_d, w1_d, w2_d, bias_d, out_d)

    nc.compile()
    return nc


def _body(tc, bass, mybir, f32, bf16, AF, OP, x_d, xo_d, qkvw_d, wbar_d,
          wbar1_d, ow_d, w1_d, w2_d, bias_d, out_d):
    nc = tc.nc
    from contextlib import ExitStack

    ctx = ExitStack()
    with ctx:
        # ---- persistent arena (stack-allocated; open for the whole kernel)
        const_pool = ctx.enter_context(tc.tile_pool(name="const", bufs=1))
        x2_pool = ctx.enter_context(tc.tile_pool(name="x2", bufs=1))
        karena = ctx.enter_context(tc.tile_pool(name="karena", bufs=1))
        kx_pool = ctx.enter_context(tc.tile_pool(name="kx", bufs=1))
        q_pool = ctx.enter_context(tc.tile_pool(name="q", bufs=1))
        v_pool = ctx.enter_context(tc.tile_pool(name="v", bufs=1))
        wqa_pool = ctx.enter_context(tc.tile_pool(name="wqa", bufs=1))
        sln_pool = ctx.enter_context(tc.tile_pool(name="sln", bufs=1))

        # raw x (bf16, feature-major) in the K-arena slots; x first in the
        # DMA queue since it gates the LN1 stats
        xb = []
        for ci in range(NCC):
            xt = karena.tile([P, T], bf16, tag=f"k{ci}", name=f"x{ci}")
            nc.sync.dma_start(xt[:], x_d[ci * P:(ci + 1) * P, :])
            xb.append(xt)

        bias_sb = const_pool.tile([P, 64], f32, tag="bias", name="bias")
        nc.sync.dma_start(bias_sb[:], bias_d[:])
        wbar_sb = const_pool.tile([1, 3 * D_MODEL], bf16, tag="wbar",
                                  name="wbar")
        nc.sync.dma_start(wbar_sb[:], wbar_d[:])
        wbar1_sb = const_pool.tile([1, D_FF], bf16, tag="wbar1", name="wbar1")
        nc.sync.dma_start(wbar1_sb[:], wbar1_d[:])
        ones_bf = const_pool.tile([P, 1], bf16, tag="ones_bf", name="ones_bf")
        nc.vector.memset(ones_bf[:], 1.0)
        ones_row = const_pool.tile([1, P], bf16, tag="ones_row",
                                   name="ones_row")
        nc.vector.memset(ones_row[:], 1.0)

        def bcol(base, i):
            return bias_sb[:, base + i:base + i + 1]
        # prefetch Q weights (arena -> no WAR on LN1 transients)
        wq = []
        for ci in range(NCC):
            wt = wqa_pool.tile([P, D_MODEL], bf16, tag=f"wq{ci}",
                               name=f"wq{ci}")
            nc.sync.dma_start(wt[:], qkvw_d[ci * P:(ci + 1) * P, 0:D_MODEL])
            wq.append(wt)

        # LN1 per-token stats: s = rsqrt(var+eps), nmu = -mu, both bf16 rows.
        # gamma/beta and the centering are folded into the projections:
        #   proj(ln(x)) = s_t * (W'.x_t + wbar.(-mu_t)) + const
        nmu_row = sln_pool.tile([1, T], bf16, tag="nmu", name="nmu_row")
        s_row = sln_pool.tile([1, T], bf16, tag="srow", name="s_row")
        s_col = sln_pool.tile([P, NKC], f32, tag="scol", name="s_col")
        nmu2_row = sln_pool.tile([1, TQ], bf16, tag="nmu2", name="nmu2_row")
        s2_bb = sln_pool.tile([P, TQ], bf16, tag="s2bb", name="s2_bb")

        with tc.tile_pool(name="xsq", bufs=2) as xsq_pool, \
             tc.tile_pool(name="ln1ps", bufs=1, space="PSUM") as lnps, \
             tc.tile_pool(name="ln1t", bufs=1) as lnt:

            st = [lnps.tile([33, 512], f32, tag=f"st{tj}", name=f"st{tj}")
                  for tj in range(NTC)]
            for ci in range(NCC):
                xsq = xsq_pool.tile([P, T], bf16, tag="xsq", name="xsq")
                nc.vector.tensor_mul(xsq[:], xb[ci][:], xb[ci][:])
                for tj in range(NTC):
                    sl = slice(tj * 512, (tj + 1) * 512)
                    nc.tensor.matmul(st[tj][0:1, :], ones_bf[:], xb[ci][:, sl],
                                     start=(ci == 0), stop=(ci == NCC - 1))
                    nc.tensor.matmul(st[tj][32:33, :], ones_bf[:], xsq[:, sl],
                                     start=(ci == 0), stop=(ci == NCC - 1))

            # drain stats to SBUF and run one batched [1, T] chain
            # (in-place ops keep the transient pool small)
            inv_n = 1.0 / D_MODEL
            ssum = lnt.tile([1, T], f32, tag="ssum", name="ssum")
            ssq = lnt.tile([1, T], f32, tag="ssq", name="ssq")
            for tj in range(NTC):
                sl = slice(tj * 512, (tj + 1) * 512)
                nc.vector.tensor_copy(ssum[:, sl], st[tj][0:1, :])
                nc.vector.tensor_copy(ssq[:, sl], st[tj][32:33, :])
            nc.vector.tensor_scalar_mul(ssum[:], ssum[:], inv_n)  # mu
            mu2 = lnt.tile([1, T], f32, tag="mu2", name="mu2")
            nc.vector.tensor_mul(mu2[:], ssum[:], ssum[:])
            nc.vector.tensor_scalar(ssq[:], ssq[:], inv_n, 1e-5,
                                    OP.mult, OP.add)
            nc.vector.tensor_sub(ssq[:], ssq[:], mu2[:])  # var + eps
            rcp1 = lnt.tile([1, T], f32, tag="mu2", name="rcp1")
            nc.vector.reciprocal_approx_fast(rcp1[:], ssq[:])
            with nc.allow_low_precision(reason="bf16 LN scale rows"):
                nc.scalar.sqrt(s_row[:], rcp1[:])
                nc.vector.tensor_scalar_mul(nmu_row[:], ssum[:], -1.0)

        # ---------------- QKV projections (from raw x) ----------------
        q_sb = [q_pool.tile([P, TQ], bf16, tag=f"q{i}", name=f"q{i}")
                for i in range(NCC)]
        k_sb = [kx_pool.tile([P, T], bf16, tag=f"kx{i}", name=f"k{i}")
                for i in range(NCC)]
        v_sb = [v_pool.tile([P, 16 * 65], bf16, tag=f"v{i}", name=f"v{i}")
                for i in range(NKC)]
        v3 = [v.rearrange("p (h s) -> p h s", s=65) for v in v_sb]

        with tc.tile_pool(name="qkvw", bufs=1) as wkv_pool, \
             tc.tile_pool(name="qkvps", bufs=6, space="PSUM") as qkv_ps, \
             tc.tile_pool(name="sbcps", bufs=2, space="PSUM") as sbc_ps, \
             tc.tile_pool(name="sbb", bufs=1) as sbb_pool:

            for tk in range(NKC):
                nc.vector.memset(v3[tk][:, :, 64:65], 1.0)

            # Q matmul chains first (they depend only on x and the stats
            # rows); the s broadcasts follow on PE and gate only the drains
            qps = []
            for co in range(NCC):
                ps = qkv_ps.tile([P, 512], f32, tag="ps", name="qkv_ps")
                for ci in range(NCC):
                    nc.tensor.matmul(ps[:], wq[ci][:, co * P:(co + 1) * P],
                                     xb[ci][:, 0:TQ], start=(ci == 0),
                                     stop=False)
                nc.tensor.matmul(ps[:], wbar_sb[:, co * P:(co + 1) * P],
                                 nmu_row[:, 0:TQ], start=False, stop=True)
                qps.append(ps)

            # per-tj broadcast of s, drained to SBUF bf16 for the Q/K drains
            sbc = []
            for tj in range(NTC):
                sl = slice(tj * 512, (tj + 1) * 512)
                sb_ps = sbc_ps.tile([P, 512], f32, tag="sb", name="sb_ps")
                nc.tensor.matmul(sb_ps[:], ones_row[:], s_row[:, sl])
                sb = sbb_pool.tile([P, 512], bf16, tag=f"sbb{tj}",
                                   name=f"sbb{tj}")
                nc.scalar.copy(sb[:], sb_ps[:])
                sbc.append(sb)

            for co in range(NCC):
                qt = x2_pool.tile([P, TQ], f32, tag=f"x2{co}", name=f"qt{co}")
                nc.vector.tensor_mul(qt[:], qps[co][:], sbc[0][:])
                nc.vector.tensor_scalar_add(q_sb[co][:], qt[:], bcol(QB, co))

            # K: [1024, 2048]; K = s * (Wk'.x + wbar_k.(-mu))
            # (the K bias shifts every score for a query equally, so it
            #  cancels in softmax and is dropped)
            wk = []
            for ci in range(NCC):
                wt = wkv_pool.tile([P, D_MODEL], bf16, tag=f"w{ci}",
                                   name=f"wk{ci}")
                nc.sync.dma_start(wt[:], qkvw_d[ci * P:(ci + 1) * P,
                                                D_MODEL:2 * D_MODEL])
                wk.append(wt)
            for co in range(NCC):
                for tj in range(NTC):
                    sl = slice(tj * 512, (tj + 1) * 512)
                    ps = qkv_ps.tile([P, 512], f32, tag="ps", name="qkv_ps")
                    for ci in range(NCC):
                        nc.tensor.matmul(ps[:],
                                         wk[ci][:, co * P:(co + 1) * P],
                                         xb[ci][:, sl], start=(ci == 0),
                                         stop=False)
                    nc.tensor.matmul(
                        ps[:], wbar_sb[:, D_MODEL + co * P:
                                       D_MODEL + (co + 1) * P],
                        nmu_row[:, sl], start=False, stop=True)
                    nc.vector.tensor_mul(k_sb[co][:, sl], ps[:], sbc[tj][:])

            # token-major s for the V drain: 16 tiny N=1 transposing matmuls
            scol_ps = sbc_ps.tile([P, NKC], f32, tag="sb", name="scol_ps")
            for tk in range(NKC):
                nc.tensor.matmul(scol_ps[:, tk:tk + 1],
                                 s_row[:, tk * P:(tk + 1) * P],
                                 ones_row[:, 0:1])
            nc.vector.tensor_copy(s_col[:], scol_ps[:])

            # V token-major with ones column; V = s_t * (x.Wv' + (-mu).wbar_v)
            wv = []
            for ci in range(NCC):
                wt = wqa_pool.tile([P, D_MODEL], bf16, tag=f"wq{ci}",
                                   name=f"wv{ci}")
                nc.sync.dma_start(wt[:], qkvw_d[ci * P:(ci + 1) * P,
                                                2 * D_MODEL:3 * D_MODEL])
                wv.append(wt)
            for tk in range(NKC):
                tsl = slice(tk * P, (tk + 1) * P)
                for vh in range(2):
                    ps = qkv_ps.tile([P, 512], f32, tag="ps", name="qkv_ps")
                    for ci in range(NCC):
                        nc.tensor.matmul(ps[:], xb[ci][:, tsl],
                                         wv[ci][:, vh * 512:(vh + 1) * 512],
                                         start=(ci == 0), stop=False)
                    nc.tensor.matmul(
                        ps[:], nmu_row[:, tsl],
                        wbar_sb[:, 2 * D_MODEL + vh * 512:
                                2 * D_MODEL + (vh + 1) * 512],
                        start=False, stop=True)
                    src = ps.rearrange("p (h d) -> p h d", d=64)
                    nc.vector.tensor_scalar_mul(
                        v3[tk][:, vh * 8:(vh + 1) * 8, 0:64], src[:],
                        s_col[:, tk:tk + 1])

        # ---------------- attention + output projection ----------------
        # Pipelined across heads: head h's scores/exp stream while head
        # h-1's PV accumulates (PE: PV(h-1) then scores(h); ACT does the
        # exps). The softmax normalize is deferred: unnormalized PV and the
        # denominator rows are drained per head, one batched reciprocal +
        # 16 broadcast matmuls normalize everything at the end.
        x2 = [x2_pool.tile([P, TQ], f32, tag=f"x2{i}", name=f"x2_{i}")
              for i in range(NCC)]
        xbc = [q_pool.tile([P, TQ], bf16, tag=f"q{i}", name=f"xb2c{i}")
               for i in range(NCC)]

        with tc.tile_pool(name="attn", bufs=1) as attn_pool:
            attn_sb = [attn_pool.tile([P, TQ], bf16, tag=f"a{i}",
                                      name=f"attn{i}") for i in range(NCC)]
            # o_w prefetch into the dead wv slots; lands early in attention
            ow = []
            for ci in range(NCC):
                wt = wqa_pool.tile([P, D_MODEL], bf16, tag=f"wq{ci}",
                                   name=f"ow{ci}")
                nc.sync.dma_start(wt[:], ow_d[ci * P:(ci + 1) * P, :])
                ow.append(wt)

            with tc.tile_pool(name="es", bufs=12) as es_pool, \
                 tc.tile_pool(name="scps", bufs=2, space="PSUM") as sc_ps, \
                 tc.tile_pool(name="pvps", bufs=2, space="PSUM") as pv_psp, \
                 tc.tile_pool(name="rbps", bufs=2, space="PSUM") as rb_psp, \
                 tc.tile_pool(name="rcp", bufs=2) as rcp_pool:

                rcf_cur = [None]

                def pv_finish(hd, pv):
                    # drain + denominator recip; normalize per finished pair
                    ct, ro = hd // 2, (hd % 2) * 64
                    ro8 = (hd % 2) * TQ
                    with nc.allow_low_precision(reason="unnormalized bf16 PV"):
                        nc.vector.tensor_copy(attn_sb[ct][ro:ro + 64, :],
                                              pv[0:64, :])
                    if hd % 2 == 0:
                        rcf_cur[0] = rcp_pool.tile([1, 2 * TQ], f32,
                                                   tag="rcf", name="recf")
                    nc.vector.reciprocal(rcf_cur[0][:, ro8:ro8 + TQ],
                                         pv[64:65, :])
                    if hd % 2 == 1:
                        recb = rcp_pool.tile([1, 2 * TQ], bf16, tag="rcb",
                                             name="recb")
                        with nc.allow_low_precision(reason="bf16 recip rows"):
                            nc.vector.tensor_copy(recb[:], rcf_cur[0][:])
                        rb = rb_psp.tile([P, TQ], f32, tag="rb", name="rb_ps")
                        nc.tensor.matmul(rb[0:64, :], ones_row[:, 0:64],
                                         recb[:, 0:TQ])
                        nc.tensor.matmul(rb[64:P, :], ones_row[:, 0:64],
                                         recb[:, TQ:2 * TQ])
                        nc.vector.tensor_mul(attn_sb[ct][:], attn_sb[ct][:],
                                             rb[:])

                # software-pipelined: head h's scores/exp stream interleaves
                # with quarter-bursts of head h-1's PV so the exp (ACT) queue
                # never starves while PE runs PV
                es_prev, es_cur = None, None
                pv_prev = None
                for hd in range(N_HEAD + 1):
                    es_prev, es_cur = es_cur, []
                    if hd < N_HEAD:
                        ct, ro = hd // 2, (hd % 2) * 64
                        ksl = k_sb[ct][ro:ro + 64, :]
                        qsl = q_sb[ct][ro:ro + 64, :]
                    if es_prev is not None:
                        pv_prev = pv_psp.tile([65, TQ], f32, tag="pv",
                                              name="pv_ps")
                    for tp in range(NKC // 2):
                        if hd < N_HEAD:
                            ps = sc_ps.tile([P, 2 * TQ], f32, tag="sc",
                                            name="sc_ps")
                            nc.tensor.matmul(ps[:, 0:TQ],
                                             ksl[:, (2 * tp) * P:
                                                 (2 * tp + 1) * P], qsl)
                            nc.tensor.matmul(ps[:, TQ:2 * TQ],
                                             ksl[:, (2 * tp + 1) * P:
                                                 (2 * tp + 2) * P], qsl)
                            e = es_pool.tile([P, 2 * TQ], bf16, tag="es",
                                             name="es")
                            nc.scalar.activation(e[:], ps[:], AF.Exp,
                                                 scale=1.0 /
                                                 np.sqrt(HEAD_DIM))
                            es_cur.append(e)
                        if es_prev is not None:
                            for tk in (2 * tp, 2 * tp + 1):
                                nc.tensor.matmul(
                                    pv_prev[:],
                                    v_sb[tk][:, (hd - 1) * 65:hd * 65],
                                    es_prev[tk // 2][:, (tk % 2) * TQ:
                                                     (tk % 2 + 1) * TQ],
                                    start=(tk == 0), stop=(tk == NKC - 1))
                    if es_prev is not None:
                        pv_finish(hd - 1, pv_prev)

            # xo borrows the v slots (v is dead after the last PV matmul)
            xo = [v_pool.tile([P, TQ], f32, tag=f"v{i}", name=f"xo{i}")
                  for i in range(NCC)]
            for ci in range(NCC):
                nc.sync.dma_start(xo[ci][:], xo_d[ci * P:(ci + 1) * P, :])

            # FFN1 first-half weights: allocate into the dead K slots now so
            # the DMAs run during the normalize/O-proj tail (per-slot WAR on
            # the last scores read; 512-col chunks land progressively)
            w1t0 = []
            for ci in range(NCC):
                wt = kx_pool.tile([P, 2048], bf16, tag=f"kx{ci}",
                                  name=f"w1t{ci}p0")
                for qc in range(4):
                    nc.sync.dma_start(
                        wt[:, qc * 512:(qc + 1) * 512],
                        w1_d[ci * P:(ci + 1) * P, qc * 512:(qc + 1) * 512])
                w1t0.append(wt)

            with tc.tile_pool(name="ops", bufs=4, space="PSUM") as o_ps:
                for co in range(NCC):
                    ps = o_ps.tile([P, TQ], f32, tag="ps", name="o_ps")
                    for hi in range(NCC):
                        nc.tensor.matmul(ps[:], ow[hi][:, co * P:(co + 1) * P],
                                         attn_sb[hi][:], start=(hi == 0),
                                         stop=(hi == NCC - 1))
                    nc.vector.scalar_tensor_tensor(x2[co][:], ps[:],
                                                   bcol(OB, co), xo[co][:],
                                                   OP.add, OP.add)
                    nc.vector.tensor_copy(xbc[co][:], x2[co][:])

        # ------- LN2 stats over x2 [1024, 512]; the normalize is folded into
        # FFN1: h1 = gelu(s2 * (W1'.x2c + wbar1.(-mu2)) + b1')
        with tc.tile_pool(name="xq2", bufs=2) as xqp, \
             tc.tile_pool(name="ln2ps", bufs=1, space="PSUM") as ln2ps, \
             tc.tile_pool(name="ln2bc", bufs=1, space="PSUM") as ln2bc, \
             tc.tile_pool(name="ln2t", bufs=2) as ln2t:
            st2 = ln2ps.tile([33, TQ], f32, tag="st2", name="st2")
            for ci in range(NCC):
                xq = xqp.tile([P, TQ], bf16, tag="xq", name="xq2")
                nc.vector.tensor_mul(xq[:], xbc[ci][:], xbc[ci][:])
                nc.tensor.matmul(st2[0:1, :], ones_bf[:], xbc[ci][:],
                                 start=(ci == 0), stop=(ci == NCC - 1))
                nc.tensor.matmul(st2[32:33, :], ones_bf[:], xq[:],
                                 start=(ci == 0), stop=(ci == NCC - 1))
            inv_n = 1.0 / D_MODEL
            mu2_sb = ln2t.tile([1, TQ], f32, tag="mu", name="mu2_sb")
            nc.vector.tensor_scalar_mul(mu2_sb[:], st2[0:1, :], inv_n)
            mu2sq = ln2t.tile([1, TQ], f32, tag="musq", name="mu2sq")
            nc.vector.tensor_mul(mu2sq[:], mu2_sb[:], mu2_sb[:])
            vpe = ln2t.tile([1, TQ], f32, tag="vpe", name="vpe2")
            nc.vector.tensor_scalar(vpe[:], st2[32:33, :], inv_n, 1e-5,
                                    OP.mult, OP.add)
            nc.vector.tensor_sub(vpe[:], vpe[:], mu2sq[:])
            rv = ln2t.tile([1, TQ], f32, tag="rv", name="rv2")
            nc.vector.reciprocal_approx_fast(rv[:], vpe[:])
            s2_f = ln2t.tile([1, TQ], f32, tag="ri", name="s2_f")
            nc.scalar.sqrt(s2_f[:], rv[:])
            with nc.allow_low_precision(reason="bf16 LN2 rows"):
                nc.vector.tensor_scalar_mul(nmu2_row[:], mu2_sb[:], -1.0)
            s2_bf = ln2t.tile([1, TQ], bf16, tag="sbf", name="s2_bf")
            nc.vector.tensor_copy(s2_bf[:], s2_f[:])
            sb_ps = ln2bc.tile([P, TQ], f32, tag="sb", name="sb2")
            nc.tensor.matmul(sb_ps[:], ones_row[:], s2_bf[:])
            nc.scalar.copy(s2_bb[:], sb_ps[:])

        # ---------------- FFN ----------------
        # h1 [4096, 512] lives in the K-arena slots as 8 groups of 4 f-chunks
        hg = [karena.tile([P, T], bf16, tag=f"k{i}", name=f"hg{i}")
              for i in range(NCC)]

        def h1sl(fch):
            return hg[fch // 4][:, (fch % 4) * 512:(fch % 4 + 1) * 512]

        with tc.tile_pool(name="h1ps", bufs=4, space="PSUM") as h1_ps, \
             tc.tile_pool(name="drt", bufs=4) as drt_pool:
            for fp in range(2):
                if fp == 0:
                    w1t = w1t0
                else:
                    w1t = []
                    for ci in range(NCC):
                        wt = kx_pool.tile([P, 2048], bf16, tag=f"kx{ci}",
                                          name=f"w1t{ci}p{fp}")
                        for qc in range(4):
                            nc.sync.dma_start(
                                wt[:, qc * 512:(qc + 1) * 512],
                                w1_d[ci * P:(ci + 1) * P,
                                     fp * 2048 + qc * 512:
                                     fp * 2048 + (qc + 1) * 512])
                        w1t.append(wt)
                for fo in range(16):
                    fch = fp * 16 + fo
                    ps = h1_ps.tile([P, TQ], f32, tag="ps", name="h1_ps")
                    for ci in range(NCC):
                        nc.tensor.matmul(ps[:],
                                         w1t[ci][:, fo * P:(fo + 1) * P],
                                         xbc[ci][:], start=(ci == 0),
                                         stop=False)
                    nc.tensor.matmul(ps[:],
                                     wbar1_sb[:, fch * P:(fch + 1) * P],
                                     nmu2_row[:], start=False, stop=True)
                    drt = drt_pool.tile([P, TQ], bf16, tag="drt", name="drt")
                    nc.vector.tensor_mul(drt[:], ps[:], s2_bb[:])
                    nc.scalar.activation(h1sl(fch), drt[:], AF.Gelu,
                                         bias=bcol(B1, fch))

        with tc.tile_pool(name="outps", bufs=1, space="PSUM") as out_ps, \
             tc.tile_pool(name="outsb", bufs=1) as out_pool:
            ops = [out_ps.tile([P, TQ], f32, tag=f"o{co}", name=f"out_ps{co}")
                   for co in range(NCC)]
            for fch in range(NFC):
                wt = wqa_pool.tile([P, D_MODEL], bf16, tag=f"wq{fch % 8}",
                                   name=f"w2t{fch}")
                nc.sync.dma_start(wt[:], w2_d[fch * P:(fch + 1) * P, :])
                for co in range(NCC):
                    nc.tensor.matmul(ops[co][:], wt[:, co * P:(co + 1) * P],
                                     h1sl(fch),
                                     start=(fch == 0), stop=(fch == NFC - 1))
            for co in range(NCC):
                osb = out_pool.tile([P, TQ], f32, tag=f"os{co}",
                                    name=f"osb{co}")
                nc.vector.scalar_tensor_tensor(osb[:], ops[co][:],
                                               bcol(B2, co), x2[co][:],
                                               OP.add, OP.add)
                nc.sync.dma_start(out_d[co * P:(co + 1) * P, :], osb[:])


def _prep_inputs(x, qkv_w, qkv_b, o_w, o_b, ln1_g, ln1_b,
                 ffn_w1, ffn_b1, ffn_w2, ffn_b2, ln2_g, ln2_b):
    import ml_dtypes
    bf = ml_dtypes.bfloat16
    f8 = np.float64

    # fold LN gammas into the following projection weights, LN betas and
    # projection biases into per-output-feature constants (data-independent)
    Wg = qkv_w.astype(f8) * ln1_g.astype(f8)[None, :]
    cvec = qkv_w.astype(f8) @ ln1_b.astype(f8) + qkv_b.astype(f8)
    qkv_wT = np.ascontiguousarray(Wg.T.astype(np.float32)).astype(bf)
    # row-sum vectors for the mean-correction rank-1 term, in bf16 to match
    # the device matmul dtype
    wbar = np.ascontiguousarray(
        Wg.sum(axis=1).astype(np.float32)[None, :]).astype(bf)
    ob_eff = (o_b.astype(f8) + o_w.astype(f8) @ cvec[2 * D_MODEL:]
              ).astype(np.float32)

    W1g = ffn_w1.astype(f8) * ln2_g.astype(f8)[None, :]
    b1_eff = (ffn_w1.astype(f8) @ ln2_b.astype(f8)
              + ffn_b1.astype(f8)).astype(np.float32)
    w1T = np.ascontiguousarray(W1g.T.astype(np.float32)).astype(bf)
    wbar1 = np.ascontiguousarray(
        W1g.sum(axis=1).astype(np.float32)[None, :]).astype(bf)

    o_wT = np.ascontiguousarray(o_w.T).astype(bf)
    w2T = np.ascontiguousarray(ffn_w2.T).astype(bf)

    def cols(v, n):
        return np.ascontiguousarray(v.reshape(n, P).T.astype(np.float32))

    biases = np.zeros((P, 64), np.float32)
    biases[:, QB:QB + 8] = cols(cvec[0:D_MODEL].astype(np.float32), 8)
    biases[:, OB:OB + 8] = cols(ob_eff, 8)
    biases[:, B1:B1 + 32] = cols(b1_eff, 32)
    biases[:, B2:B2 + 8] = cols(ffn_b2, 8)

    in_maps = []
    for c in range(N_CORES):
        b, s = c // GROUPS, c % GROUPS
        xr = np.ascontiguousarray(np.roll(x[b], -s * TQ, axis=0).T)
        in_maps.append({
            "x_fm": xr.astype(bf),
            "x_own": np.ascontiguousarray(xr[:, :TQ]),
            "qkv_wT": qkv_wT,
            "wbar": wbar,
            "wbar1": wbar1,
            "o_wT": o_wT,
            "w1T": w1T,
            "w2T": w2T,
            "biases": biases,
        })
    return in_maps


def kernel(**inputs):
    from concourse.bass_utils import run_bass_kernel_spmd

    if "nc" not in _cache:
        _cache["nc"] = _build()
    nc = _cache["nc"]

    inputs = {k: np.asarray(v, dtype=np.float32) for k, v in inputs.items()}
    in_maps = _prep_inputs(**inputs)

    res = run_bass_kernel_spmd(nc, in_maps, core_ids=list(range(N_CORES)),
                               **_cache.get("run_kwargs", {}))
    _cache["last_results"] = res

    out = np.empty((B, T, D_MODEL), np.float32)
    for c in range(N_CORES):
        b, s = c // GROUPS, c % GROUPS
        out[b, s * TQ:(s + 1) * TQ, :] = res.results[c]["out"].T
    return out



# revision 16
# speedup vs baseline: 1.0224x; 1.0224x over previous
"""Trainium2 Bass kernel for a minimal transformer block (B=2, T=2048, C=1024,
H=16, Dh=64, F=4096), sharded over 8 NeuronCores.

Sharding: data-parallel over batch (2 groups of 4 cores) x sequence-parallel
over tokens within each batch (512 query tokens per core). Each core
redundantly computes K/V for its batch's full sequence, which removes all
cross-core communication: the host only concatenates the per-core output
slices. Per-core token identity is established by rolling the sequence so the
core's own 512 tokens come first (softmax attention without a mask is
invariant to key/value ordering).

Everything on-chip is feature-major ([features, tokens]); the host transposes
inputs/outputs and pre-transposes/casts weights to bf16.
"""

import sys

if "/opt/trn_rl_repo" not in sys.path:
    sys.path.insert(0, "/opt/trn_rl_repo")

import numpy as np

D_MODEL = 1024
N_HEAD = 16
HEAD_DIM = 64
D_FF = 4096
B = 2
T = 2048
N_CORES = 8
GROUPS = 4          # cores per batch
TQ = T // GROUPS    # own query tokens per core = 512
P = 128
NCC = D_MODEL // P  # 8 C-chunks
NTC = T // 512      # 4 T-chunks of 512
NKC = T // P        # 16 k-chunks of 128
NFC = D_FF // P     # 32 f-chunks of 128

# bias-table column layout ([128, 64] f32)
QB, KB, OB, B1, B2 = 0, 8, 16, 24, 56

_cache = {}


def _build():
    import concourse.bass as bass
    import concourse.tile as tile
    from concourse import bacc, mybir

    f32 = mybir.dt.float32
    bf16 = mybir.dt.bfloat16
    AF = mybir.ActivationFunctionType
    OP = mybir.AluOpType

    nc = bacc.Bacc("TRN2", target_bir_lowering=False, debug=False,
                   num_devices=N_CORES)

    x_d = nc.dram_tensor("x_fm", [D_MODEL, T], bf16, kind="ExternalInput").ap()
    xo_d = nc.dram_tensor("x_own", [D_MODEL, TQ], f32,
                          kind="ExternalInput").ap()
    qkvw_d = nc.dram_tensor("qkv_wT", [D_MODEL, 3 * D_MODEL], bf16,
                            kind="ExternalInput").ap()
    wbar_d = nc.dram_tensor("wbar", [1, 3 * D_MODEL], bf16,
                            kind="ExternalInput").ap()
    wbar1_d = nc.dram_tensor("wbar1", [1, D_FF], bf16,
                             kind="ExternalInput").ap()
    ow_d = nc.dram_tensor("o_wT", [D_MODEL, D_MODEL], bf16,
                          kind="ExternalInput").ap()
    w1_d = nc.dram_tensor("w1T", [D_MODEL, D_FF], bf16,
                          kind="ExternalInput").ap()
    w2_d = nc.dram_tensor("w2T", [D_FF, D_MODEL], bf16,
                          kind="ExternalInput").ap()
    bias_d = nc.dram_tensor("biases", [P, 64], f32, kind="ExternalInput").ap()
    out_d = nc.dram_tensor("out", [D_MODEL, TQ], f32,
                           kind="ExternalOutput").ap()

    with tile.TileContext(nc) as tc:
        _body(tc, bass, mybir, f32, bf16, AF, OP, x_d, xo_d, qkvw_d, wbar_d,
              wbar1_d, ow_d, w1_d, w2_d, bias_d, out_d)

    nc.compile()
    return nc


def _body(tc, bass, mybir, f32, bf16, AF, OP, x_d, xo_d, qkvw_d, wbar_d,
          wbar1_d, ow_d, w1_d, w2_d, bias_d, out_d):
    nc = tc.nc
    from contextlib import ExitStack

    ctx = ExitStack()
    with ctx:
        # ---- persistent arena (stack-allocated; open for the whole kernel)
        const_pool = ctx.enter_context(tc.tile_pool(name="const", bufs=1))
        x2_pool = ctx.enter_context(tc.tile_pool(name="x2", bufs=1))
        karena = ctx.enter_context(tc.tile_pool(name="karena", bufs=1))
        kx_pool = ctx.enter_context(tc.tile_pool(name="kx", bufs=1))
        q_pool = ctx.enter_context(tc.tile_pool(name="q", bufs=1))
        v_pool = ctx.enter_context(tc.tile_pool(name="v", bufs=1))
        wqa_pool = ctx.enter_context(tc.tile_pool(name="wqa", bufs=1))
        sln_pool = ctx.enter_context(tc.tile_pool(name="sln", bufs=1))

        # raw x (bf16, feature-major) in the K-arena slots; x first in the
        # DMA queue since it gates the LN1 stats
        xb = []
        for ci in range(NCC):
            xt = karena.tile([P, T], bf16, tag=f"k{ci}", name=f"x{ci}")
            nc.sync.dma_start(xt[:], x_d[ci * P:(ci + 1) * P, :])
            xb.append(xt)

        bias_sb = const_pool.tile([P, 64], f32, tag="bias", name="bias")
        nc.sync.dma_start(bias_sb[:], bias_d[:])
        wbar_sb = const_pool.tile([1, 3 * D_MODEL], bf16, tag="wbar",
                                  name="wbar")
        nc.sync.dma_start(wbar_sb[:], wbar_d[:])
        wbar1_sb = const_pool.tile([1, D_FF], bf16, tag="wbar1", name="wbar1")
        nc.sync.dma_start(wbar1_sb[:], wbar1_d[:])
        ones_bf = const_pool.tile([P, 1], bf16, tag="ones_bf", name="ones_bf")
        nc.vector.memset(ones_bf[:], 1.0)
        ones_row = const_pool.tile([1, P], bf16, tag="ones_row",
                                   name="ones_row")
        nc.vector.memset(ones_row[:], 1.0)

        def bcol(base, i):
            return bias_sb[:, base + i:base + i + 1]
        # prefetch Q weights (arena -> no WAR on LN1 transients)
        wq = []
        for ci in range(NCC):
            wt = wqa_pool.tile([P, D_MODEL], bf16, tag=f"wq{ci}",
                               name=f"wq{ci}")
            nc.sync.dma_start(wt[:], qkvw_d[ci * P:(ci + 1) * P, 0:D_MODEL])
            wq.append(wt)

        # LN1 per-token stats: s = rsqrt(var+eps), nmu = -mu, both bf16 rows.
        # gamma/beta and the centering are folded into the projections:
        #   proj(ln(x)) = s_t * (W'.x_t + wbar.(-mu_t)) + const
        nmu_row = sln_pool.tile([1, T], bf16, tag="nmu", name="nmu_row")
        s_row = sln_pool.tile([1, T], bf16, tag="srow", name="s_row")
        s_col = sln_pool.tile([P, NKC], f32, tag="scol", name="s_col")
        nmu2_row = sln_pool.tile([1, TQ], bf16, tag="nmu2", name="nmu2_row")
        s2_bb = sln_pool.tile([P, TQ], bf16, tag="s2bb", name="s2_bb")

        with tc.tile_pool(name="xsq", bufs=2) as xsq_pool, \
             tc.tile_pool(name="ln1ps", bufs=1, space="PSUM") as lnps, \
             tc.tile_pool(name="ln1t", bufs=1) as lnt:

            st = [lnps.tile([33, 512], f32, tag=f"st{tj}", name=f"st{tj}")
                  for tj in range(NTC)]
            for ci in range(NCC):
                xsq = xsq_pool.tile([P, T], bf16, tag="xsq", name="xsq")
                nc.vector.tensor_mul(xsq[:], xb[ci][:], xb[ci][:])
                for tj in range(NTC):
                    sl = slice(tj * 512, (tj + 1) * 512)
                    nc.tensor.matmul(st[tj][0:1, :], ones_bf[:], xb[ci][:, sl],
                                     start=(ci == 0), stop=(ci == NCC - 1))
                    nc.tensor.matmul(st[tj][32:33, :], ones_bf[:], xsq[:, sl],
                                     start=(ci == 0), stop=(ci == NCC - 1))

            # drain stats to SBUF and run one batched [1, T] chain
            # (in-place ops keep the transient pool small)
            inv_n = 1.0 / D_MODEL
            ssum = lnt.tile([1, T], f32, tag="ssum", name="ssum")
            ssq = lnt.tile([1, T], f32, tag="ssq", name="ssq")
            for tj in range(NTC):
                sl = slice(tj * 512, (tj + 1) * 512)
                nc.vector.tensor_copy(ssum[:, sl], st[tj][0:1, :])
                nc.vector.tensor_copy(ssq[:, sl], st[tj][32:33, :])
            nc.vector.tensor_scalar_mul(ssum[:], ssum[:], inv_n)  # mu
            mu2 = lnt.tile([1, T], f32, tag="mu2", name="mu2")
            nc.vector.tensor_mul(mu2[:], ssum[:], ssum[:])
            nc.vector.tensor_scalar(ssq[:], ssq[:], inv_n, 1e-5,
                                    OP.mult, OP.add)
            nc.vector.tensor_sub(ssq[:], ssq[:], mu2[:])  # var + eps
            rcp1 = lnt.tile([1, T], f32, tag="mu2", name="rcp1")
            nc.vector.reciprocal_approx_fast(rcp1[:], ssq[:])
            with nc.allow_low_precision(reason="bf16 LN scale rows"):
                nc.scalar.sqrt(s_row[:], rcp1[:])
                nc.vector.tensor_scalar_mul(nmu_row[:], ssum[:], -1.0)

        # ---------------- QKV projections (from raw x) ----------------
        q_sb = [q_pool.tile([P, TQ], bf16, tag=f"q{i}", name=f"q{i}")
                for i in range(NCC)]
        k_sb = [kx_pool.tile([P, T], bf16, tag=f"kx{i}", name=f"k{i}")
                for i in range(NCC)]
        v_sb = [v_pool.tile([P, 16 * 65], bf16, tag=f"v{i}", name=f"v{i}")
                for i in range(NKC)]
        v3 = [v.rearrange("p (h s) -> p h s", s=65) for v in v_sb]

        with tc.tile_pool(name="qkvw", bufs=1) as wkv_pool, \
             tc.tile_pool(name="qkvps", bufs=6, space="PSUM") as qkv_ps, \
             tc.tile_pool(name="sbcps", bufs=2, space="PSUM") as sbc_ps, \
             tc.tile_pool(name="sbb", bufs=1) as sbb_pool:

            for tk in range(NKC):
                nc.vector.memset(v3[tk][:, :, 64:65], 1.0)

            # Q matmul chains first (they depend only on x and the stats
            # rows); the s broadcasts follow on PE and gate only the drains
            qps = []
            for co in range(NCC):
                ps = qkv_ps.tile([P, 512], f32, tag="ps", name="qkv_ps")
                for ci in range(NCC):
                    nc.tensor.matmul(ps[:], wq[ci][:, co * P:(co + 1) * P],
                                     xb[ci][:, 0:TQ], start=(ci == 0),
                                     stop=False)
                nc.tensor.matmul(ps[:], wbar_sb[:, co * P:(co + 1) * P],
                                 nmu_row[:, 0:TQ], start=False, stop=True)
                qps.append(ps)

            # per-tj broadcast of s, drained to SBUF bf16 for the Q/K drains
            sbc = []
            for tj in range(NTC):
                sl = slice(tj * 512, (tj + 1) * 512)
                sb_ps = sbc_ps.tile([P, 512], f32, tag="sb", name="sb_ps")
                nc.tensor.matmul(sb_ps[:], ones_row[:], s_row[:, sl])
                sb = sbb_pool.tile([P, 512], bf16, tag=f"sbb{tj}",
                                   name=f"sbb{tj}")
                nc.scalar.copy(sb[:], sb_ps[:])
                sbc.append(sb)

            for co in range(NCC):
                qt = x2_pool.tile([P, TQ], f32, tag=f"x2{co}", name=f"qt{co}")
                nc.vector.tensor_mul(qt[:], qps[co][:], sbc[0][:])
                nc.vector.tensor_scalar_add(q_sb[co][:], qt[:], bcol(QB, co))

            # K: [1024, 2048]; K = s * (Wk'.x + wbar_k.(-mu))
            # (the K bias shifts every score for a query equally, so it
            #  cancels in softmax and is dropped)
            wk = []
            for ci in range(NCC):
                wt = wkv_pool.tile([P, D_MODEL], bf16, tag=f"w{ci}",
                                   name=f"wk{ci}")
                nc.sync.dma_start(wt[:], qkvw_d[ci * P:(ci + 1) * P,
                                                D_MODEL:2 * D_MODEL])
                wk.append(wt)
            for co in range(NCC):
                for tj in range(NTC):
                    sl = slice(tj * 512, (tj + 1) * 512)
                    ps = qkv_ps.tile([P, 512], f32, tag="ps", name="qkv_ps")
                    for ci in range(NCC):
                        nc.tensor.matmul(ps[:],
                                         wk[ci][:, co * P:(co + 1) * P],
                                         xb[ci][:, sl], start=(ci == 0),
                                         stop=False)
                    nc.tensor.matmul(
                        ps[:], wbar_sb[:, D_MODEL + co * P:
                                       D_MODEL + (co + 1) * P],
                        nmu_row[:, sl], start=False, stop=True)
                    nc.vector.tensor_mul(k_sb[co][:, sl], ps[:], sbc[tj][:])

            # token-major s for the V drain: 16 tiny N=1 transposing matmuls
            scol_ps = sbc_ps.tile([P, NKC], f32, tag="sb", name="scol_ps")
            for tk in range(NKC):
                nc.tensor.matmul(scol_ps[:, tk:tk + 1],
                                 s_row[:, tk * P:(tk + 1) * P],
                                 ones_row[:, 0:1])
            nc.vector.tensor_copy(s_col[:], scol_ps[:])

            # V token-major with ones column; V = s_t * (x.Wv' + (-mu).wbar_v)
            wv = []
            for ci in range(NCC):
                wt = wqa_pool.tile([P, D_MODEL], bf16, tag=f"wq{ci}",
                                   name=f"wv{ci}")
                nc.sync.dma_start(wt[:], qkvw_d[ci * P:(ci + 1) * P,
                                                2 * D_MODEL:3 * D_MODEL])
                wv.append(wt)
            for tk in range(NKC):
                tsl = slice(tk * P, (tk + 1) * P)
                for vh in range(2):
                    ps = qkv_ps.tile([P, 512], f32, tag="ps", name="qkv_ps")
                    for ci in range(NCC):
                        nc.tensor.matmul(ps[:], xb[ci][:, tsl],
                                         wv[ci][:, vh * 512:(vh + 1) * 512],
                                         start=(ci == 0), stop=False)
                    nc.tensor.matmul(
                        ps[:], nmu_row[:, tsl],
                        wbar_sb[:, 2 * D_MODEL + vh * 512:
                                2 * D_MODEL + (vh + 1) * 512],
                        start=False, stop=True)
                    src = ps.rearrange("p (h d) -> p h d", d=64)
                    nc.vector.tensor_scalar_mul(
                        v3[tk][:, vh * 8:(vh + 1) * 8, 0:64], src[:],
                        s_col[:, tk:tk + 1])

        # ---------------- attention + output projection ----------------
        # Pipelined across heads: head h's scores/exp stream while head
        # h-1's PV accumulates (PE: PV(h-1) then scores(h); ACT does the
        # exps). The softmax normalize is deferred: unnormalized PV and the
        # denominator rows are drained per head, one batched reciprocal +
        # 16 broadcast matmuls normalize everything at the end.
        x2 = [x2_pool.tile([P, TQ], f32, tag=f"x2{i}", name=f"x2_{i}")
              for i in range(NCC)]
        xbc = [q_pool.tile([P, TQ], bf16, tag=f"q{i}", name=f"xb2c{i}")
               for i in range(NCC)]

        with tc.tile_pool(name="attn", bufs=1) as attn_pool:
            attn_sb = [attn_pool.tile([P, TQ], bf16, tag=f"a{i}",
                                      name=f"attn{i}") for i in range(NCC)]
            # o_w prefetch into the dead wv slots; lands early in attention
            ow = []
            for ci in range(NCC):
                wt = wqa_pool.tile([P, D_MODEL], bf16, tag=f"wq{ci}",
                                   name=f"ow{ci}")
                nc.sync.dma_start(wt[:], ow_d[ci * P:(ci + 1) * P, :])
                ow.append(wt)

            with tc.tile_pool(name="es", bufs=12) as es_pool, \
                 tc.tile_pool(name="scps", bufs=2, space="PSUM") as sc_ps, \
                 tc.tile_pool(name="pvps", bufs=2, space="PSUM") as pv_psp, \
                 tc.tile_pool(name="rbps", bufs=2, space="PSUM") as rb_psp, \
                 tc.tile_pool(name="rcp", bufs=2) as rcp_pool:

                rcf_cur = [None]

                def pv_finish(hd, pv):
                    # drain + denominator recip; normalize per finished pair
                    ct, ro = hd // 2, (hd % 2) * 64
                    ro8 = (hd % 2) * TQ
                    with nc.allow_low_precision(reason="unnormalized bf16 PV"):
                        nc.vector.tensor_copy(attn_sb[ct][ro:ro + 64, :],
                                              pv[0:64, :])
                    if hd % 2 == 0:
                        rcf_cur[0] = rcp_pool.tile([1, 2 * TQ], f32,
                                                   tag="rcf", name="recf")
                    nc.vector.reciprocal(rcf_cur[0][:, ro8:ro8 + TQ],
                                         pv[64:65, :])
                    if hd % 2 == 1:
                        recb = rcp_pool.tile([1, 2 * TQ], bf16, tag="rcb",
                                             name="recb")
                        with nc.allow_low_precision(reason="bf16 recip rows"):
                            nc.vector.tensor_copy(recb[:], rcf_cur[0][:])
                        rb = rb_psp.tile([P, TQ], f32, tag="rb", name="rb_ps")
                        nc.tensor.matmul(rb[0:64, :], ones_row[:, 0:64],
                                         recb[:, 0:TQ])
                        nc.tensor.matmul(rb[64:P, :], ones_row[:, 0:64],
                                         recb[:, TQ:2 * TQ])
                        nc.vector.tensor_mul(attn_sb[ct][:], attn_sb[ct][:],
                                             rb[:])

                # software-pipelined: 4 score-pairs of head h, then head
                # h-1's full PV burst (ACT drains the queued exps during
                # it), then the last 4 score-pairs
                def sc_pair(hd, tp, ksl, qsl, es_cur):
                    ps = sc_ps.tile([P, 2 * TQ], f32, tag="sc", name="sc_ps")
                    nc.tensor.matmul(ps[:, 0:TQ],
                                     ksl[:, (2 * tp) * P:(2 * tp + 1) * P],
                                     qsl)
                    nc.tensor.matmul(ps[:, TQ:2 * TQ],
                                     ksl[:, (2 * tp + 1) * P:
                                         (2 * tp + 2) * P], qsl)
                    e = es_pool.tile([P, 2 * TQ], bf16, tag="es", name="es")
                    nc.scalar.activation(e[:], ps[:], AF.Exp,
                                         scale=1.0 / np.sqrt(HEAD_DIM))
                    es_cur.append(e)

                es_prev, es_cur = None, None
                for hd in range(N_HEAD + 1):
                    es_prev, es_cur = es_cur, []
                    if hd < N_HEAD:
                        ct, ro = hd // 2, (hd % 2) * 64
                        ksl = k_sb[ct][ro:ro + 64, :]
                        qsl = q_sb[ct][ro:ro + 64, :]
                        for tp in range(4):
                            sc_pair(hd, tp, ksl, qsl, es_cur)
                    if es_prev is not None:
                        pv_prev = pv_psp.tile([65, TQ], f32, tag="pv",
                                              name="pv_ps")
                        for tk in range(NKC):
                            nc.tensor.matmul(
                                pv_prev[:],
                                v_sb[tk][:, (hd - 1) * 65:hd * 65],
                                es_prev[tk // 2][:, (tk % 2) * TQ:
                                                 (tk % 2 + 1) * TQ],
                                start=(tk == 0), stop=(tk == NKC - 1))
                    if hd < N_HEAD:
                        for tp in range(4, NKC // 2):
                            sc_pair(hd, tp, ksl, qsl, es_cur)
                    if es_prev is not None:
                        pv_finish(hd - 1, pv_prev)

            # xo borrows the v slots (v is dead after the last PV matmul)
            xo = [v_pool.tile([P, TQ], f32, tag=f"v{i}", name=f"xo{i}")
                  for i in range(NCC)]
            for ci in range(NCC):
                nc.sync.dma_start(xo[ci][:], xo_d[ci * P:(ci + 1) * P, :])

            # FFN1 first-half weights: allocate into the dead K slots now so
            # the DMAs run during the normalize/O-proj tail (per-slot WAR on
            # the last scores read; 512-col chunks land progressively)
            w1t0 = []
            for ci in range(NCC):
                wt = kx_pool.tile([P, 2048], bf16, tag=f"kx{ci}",
                                  name=f"w1t{ci}p0")
                for qc in range(4):
                    nc.sync.dma_start(
                        wt[:, qc * 512:(qc + 1) * 512],
                        w1_d[ci * P:(ci + 1) * P, qc * 512:(qc + 1) * 512])
                w1t0.append(wt)

            with tc.tile_pool(name="ops", bufs=4, space="PSUM") as o_ps:
                for co in range(NCC):
                    ps = o_ps.tile([P, TQ], f32, tag="ps", name="o_ps")
                    for hi in range(NCC):
                        nc.tensor.matmul(ps[:], ow[hi][:, co * P:(co + 1) * P],
                                         attn_sb[hi][:], start=(hi == 0),
                                         stop=(hi == NCC - 1))
                    nc.vector.scalar_tensor_tensor(x2[co][:], ps[:],
                                                   bcol(OB, co), xo[co][:],
                                                   OP.add, OP.add)
                    nc.vector.tensor_copy(xbc[co][:], x2[co][:])

        # ------- LN2 stats over x2 [1024, 512]; the normalize is folded into
        # FFN1: h1 = gelu(s2 * (W1'.x2c + wbar1.(-mu2)) + b1')
        with tc.tile_pool(name="xq2", bufs=2) as xqp, \
             tc.tile_pool(name="ln2ps", bufs=1, space="PSUM") as ln2ps, \
             tc.tile_pool(name="ln2bc", bufs=1, space="PSUM") as ln2bc, \
             tc.tile_pool(name="ln2t", bufs=2) as ln2t:
            st2 = ln2ps.tile([33, TQ], f32, tag="st2", name="st2")
            for ci in range(NCC):
                xq = xqp.tile([P, TQ], bf16, tag="xq", name="xq2")
                nc.vector.tensor_mul(xq[:], xbc[ci][:], xbc[ci][:])
                nc.tensor.matmul(st2[0:1, :], ones_bf[:], xbc[ci][:],
                                 start=(ci == 0), stop=(ci == NCC - 1))
                nc.tensor.matmul(st2[32:33, :], ones_bf[:], xq[:],
                                 start=(ci == 0), stop=(ci == NCC - 1))
            inv_n = 1.0 / D_MODEL
            mu2_sb = ln2t.tile([1, TQ], f32, tag="mu", name="mu2_sb")
            nc.vector.tensor_scalar_mul(mu2_sb[:], st2[0:1, :], inv_n)
            mu2sq = ln2t.tile([1, TQ], f32, tag="musq", name="mu2sq")
            nc.vector.tensor_mul(mu2sq[:], mu2_sb[:], mu2_sb[:])
            vpe = ln2t.tile([1, TQ], f32, tag="vpe", name="vpe2")
            nc.vector.tensor_scalar(vpe[:], st2[32:33, :], inv_n, 1e-5,
                                    OP.mult, OP.add)
            nc.vector.tensor_sub(vpe[:], vpe[:], mu2sq[:])
            rv = ln2t.tile([1, TQ], f32, tag="rv", name="rv2")
            nc.vector.reciprocal_approx_fast(rv[:], vpe[:])
            s2_f = ln2t.tile([1, TQ], f32, tag="ri", name="s2_f")
            nc.scalar.sqrt(s2_f[:], rv[:])
            with nc.allow_low_precision(reason="bf16 LN2 rows"):
                nc.vector.tensor_scalar_mul(nmu2_row[:], mu2_sb[:], -1.0)
            s2_bf = ln2t.tile([1, TQ], bf16, tag="sbf", name="s2_bf")
            nc.vector.tensor_copy(s2_bf[:], s2_f[:])
            sb_ps = ln2bc.tile([P, TQ], f32, tag="sb", name="sb2")
            nc.tensor.matmul(sb_ps[:], ones_row[:], s2_bf[:])
            nc.scalar.copy(s2_bb[:], sb_ps[:])

        # ---------------- FFN ----------------
        # h1 [4096, 512] lives in the K-arena slots as 8 groups of 4 f-chunks
        hg = [karena.tile([P, T], bf16, tag=f"k{i}", name=f"hg{i}")
              for i in range(NCC)]

        def h1sl(fch):
            return hg[fch // 4][:, (fch % 4) * 512:(fch % 4 + 1) * 512]

        with tc.tile_pool(name="h1ps", bufs=4, space="PSUM") as h1_ps, \
             tc.tile_pool(name="drt", bufs=4) as drt_pool:
            for fp in range(2):
                if fp == 0:
                    w1t = w1t0
                else:
                    w1t = []
                    for ci in range(NCC):
                        wt = kx_pool.tile([P, 2048], bf16, tag=f"kx{ci}",
                                          name=f"w1t{ci}p{fp}")
                        for qc in range(4):
                            nc.sync.dma_start(
                                wt[:, qc * 512:(qc + 1) * 512],
                                w1_d[ci * P:(ci + 1) * P,
                                     fp * 2048 + qc * 512:
                                     fp * 2048 + (qc + 1) * 512])
                        w1t.append(wt)
                for fo in range(16):
                    fch = fp * 16 + fo
                    ps = h1_ps.tile([P, TQ], f32, tag="ps", name="h1_ps")
                    for ci in range(NCC):
                        nc.tensor.matmul(ps[:],
                                         w1t[ci][:, fo * P:(fo + 1) * P],
                                         xbc[ci][:], start=(ci == 0),
                                         stop=False)
                    nc.tensor.matmul(ps[:],
                                     wbar1_sb[:, fch * P:(fch + 1) * P],
                                     nmu2_row[:], start=False, stop=True)
                    drt = drt_pool.tile([P, TQ], bf16, tag="drt", name="drt")
                    nc.vector.tensor_mul(drt[:], ps[:], s2_bb[:])
                    nc.scalar.activation(h1sl(fch), drt[:], AF.Gelu,
                                         bias=bcol(B1, fch))

        with tc.tile_pool(name="outps", bufs=1, space="PSUM") as out_ps, \
             tc.tile_pool(name="outsb", bufs=1) as out_pool:
            ops = [out_ps.tile([P, TQ], f32, tag=f"o{co}", name=f"out_ps{co}")
                   for co in range(NCC)]
            for fch in range(NFC):
                wt = wqa_pool.tile([P, D_MODEL], bf16, tag=f"wq{fch % 8}",
                                   name=f"w2t{fch}")
                nc.sync.dma_start(wt[:], w2_d[fch * P:(fch + 1) * P, :])
                for co in range(NCC):
                    nc.tensor.matmul(ops[co][:], wt[:, co * P:(co + 1) * P],
                                     h1sl(fch),
                                     start=(fch == 0), stop=(fch == NFC - 1))
            for co in range(NCC):
                osb = out_pool.tile([P, TQ], f32, tag=f"os{co}",
                                    name=f"osb{co}")
                nc.vector.scalar_tensor_tensor(osb[:], ops[co][:],
                                               bcol(B2, co), x2[co][:],
                                               OP.add, OP.add)
                nc.sync.dma_start(out_d[co * P:(co + 1) * P, :], osb[:])


def _prep_inputs(x, qkv_w, qkv_b, o_w, o_b, ln1_g, ln1_b,
                 ffn_w1, ffn_b1, ffn_w2, ffn_b2, ln2_g, ln2_b):
    import ml_dtypes
    bf = ml_dtypes.bfloat16
    f8 = np.float64

    # fold LN gammas into the following projection weights, LN betas and
    # projection biases into per-output-feature constants (data-independent)
    Wg = qkv_w.astype(f8) * ln1_g.astype(f8)[None, :]
    cvec = qkv_w.astype(f8) @ ln1_b.astype(f8) + qkv_b.astype(f8)
    qkv_wT = np.ascontiguousarray(Wg.T.astype(np.float32)).astype(bf)
    # row-sum vectors for the mean-correction rank-1 term, in bf16 to match
    # the device matmul dtype
    wbar = np.ascontiguousarray(
        Wg.sum(axis=1).astype(np.float32)[None, :]).astype(bf)
    ob_eff = (o_b.astype(f8) + o_w.astype(f8) @ cvec[2 * D_MODEL:]
              ).astype(np.float32)

    W1g = ffn_w1.astype(f8) * ln2_g.astype(f8)[None, :]
    b1_eff = (ffn_w1.astype(f8) @ ln2_b.astype(f8)
              + ffn_b1.astype(f8)).astype(np.float32)
    w1T = np.ascontiguousarray(W1g.T.astype(np.float32)).astype(bf)
    wbar1 = np.ascontiguousarray(
        W1g.sum(axis=1).astype(np.float32)[None, :]).astype(bf)

    o_wT = np.ascontiguousarray(o_w.T).astype(bf)
    w2T = np.ascontiguousarray(ffn_w2.T).astype(bf)

    def cols(v, n):
        return np.ascontiguousarray(v.reshape(n, P).T.astype(np.float32))

    biases = np.zeros((P, 64), np.float32)
    biases[:, QB:QB + 8] = cols(cvec[0:D_MODEL].astype(np.float32), 8)
    biases[:, OB:OB + 8] = cols(ob_eff, 8)
    biases[:, B1:B1 + 32] = cols(b1_eff, 32)
    biases[:, B2:B2 + 8] = cols(ffn_b2, 8)

    in_maps = []
    for c in range(N_CORES):
        b, s = c // GROUPS, c % GROUPS
        xr = np.ascontiguousarray(np.roll(x[b], -s * TQ, axis=0).T)
        in_maps.append({
            "x_fm": xr.astype(bf),
            "x_own": np.ascontiguousarray(xr[:, :TQ]),
            "qkv_wT": qkv_wT,
            "wbar": wbar,
            "wbar1": wbar1,
            "o_wT": o_wT,
            "w1T": w1T,
            "w2T": w2T,
            "biases": biases,
        })
    return in_maps


def kernel(**inputs):
    from concourse.bass_utils import run_bass_kernel_spmd

    if "nc" not in _cache:
        _cache["nc"] = _build()
    nc = _cache["nc"]

    inputs = {k: np.asarray(v, dtype=np.float32) for k, v in inputs.items()}
    in_maps = _prep_inputs(**inputs)

    res = run_bass_kernel_spmd(nc, in_maps, core_ids=list(range(N_CORES)),
                               **_cache.get("run_kwargs", {}))
    _cache["last_results"] = res

    out = np.empty((B, T, D_MODEL), np.float32)
    for c in range(N_CORES):
        b, s = c // GROUPS, c % GROUPS
        out[b, s * TQ:(s + 1) * TQ, :] = res.results[c]["out"].T
    return out



# revision 19
# speedup vs baseline: 1.0303x; 1.0077x over previous
"""Trainium2 Bass kernel for a minimal transformer block (B=2, T=2048, C=1024,
H=16, Dh=64, F=4096), sharded over 8 NeuronCores.

Sharding: data-parallel over batch (2 groups of 4 cores) x sequence-parallel
over tokens within each batch (512 tokens per core). Each core computes
Q/K/V only for its own 512 tokens from a pre-normalized activation
xln = (x - mu) * rsqrt(var + eps) (LN gammas/betas folded into the weights /
bias table on the host); K and V are then AllGathered across the 4 cores of
each batch group so every core attends over the full 2048 keys. Keys are kept
in global token order (softmax without a mask is permutation-invariant).

Everything on-chip is feature-major ([features, tokens]); the host transposes
inputs/outputs and pre-transposes/casts weights to bf16.
"""

import sys

if "/opt/trn_rl_repo" not in sys.path:
    sys.path.insert(0, "/opt/trn_rl_repo")

import numpy as np

D_MODEL = 1024
N_HEAD = 16
HEAD_DIM = 64
D_FF = 4096
B = 2
T = 2048
N_CORES = 8
GROUPS = 4          # cores per batch
TQ = T // GROUPS    # own tokens per core = 512
P = 128
NCC = D_MODEL // P  # 8 C-chunks
NKC = T // P        # 16 k-chunks of 128
NTK = TQ // P       # 4 own-token chunks of 128
NFC = D_FF // P     # 32 f-chunks of 128

# bias-table column layout ([128, 64] f32)
QB, KB, OB, B1, B2 = 0, 8, 16, 24, 56

_cache = {}


def _build():
    import concourse.bass as bass
    import concourse.tile as tile
    from concourse import bacc, mybir

    f32 = mybir.dt.float32
    bf16 = mybir.dt.bfloat16
    AF = mybir.ActivationFunctionType
    OP = mybir.AluOpType

    nc = bacc.Bacc("TRN2", target_bir_lowering=False, debug=False,
                   num_devices=N_CORES)

    x_d = nc.dram_tensor("x_fm", [D_MODEL, TQ], bf16,
                         kind="ExternalInput").ap()
    xo_d = nc.dram_tensor("x_own", [D_MODEL, TQ], f32,
                          kind="ExternalInput").ap()
    qkvw_d = nc.dram_tensor("qkv_wT", [D_MODEL, 3 * D_MODEL], bf16,
                            kind="ExternalInput").ap()
    ow_d = nc.dram_tensor("o_wT", [D_MODEL, D_MODEL], bf16,
                          kind="ExternalInput").ap()
    w1_d = nc.dram_tensor("w1T", [D_MODEL, D_FF], bf16,
                          kind="ExternalInput").ap()
    w2_d = nc.dram_tensor("w2T", [D_FF, D_MODEL], bf16,
                          kind="ExternalInput").ap()
    bias_d = nc.dram_tensor("biases", [P, 64], f32, kind="ExternalInput").ap()
    out_d = nc.dram_tensor("out", [D_MODEL, TQ], f32,
                           kind="ExternalOutput").ap()

    with tile.TileContext(nc) as tc:
        _body(tc, bass, mybir, f32, bf16, AF, OP, x_d, xo_d, qkvw_d,
              ow_d, w1_d, w2_d, bias_d, out_d)

    nc.compile()
    return nc


def _body(tc, bass, mybir, f32, bf16, AF, OP, x_d, xo_d, qkvw_d,
          ow_d, w1_d, w2_d, bias_d, out_d):
    nc = tc.nc
    from contextlib import ExitStack

    RG = [[0, 1, 2, 3], [4, 5, 6, 7]]

    ctx = ExitStack()
    with ctx:
        # ---- persistent arena (stack-allocated; open for the whole kernel)
        const_pool = ctx.enter_context(tc.tile_pool(name="const", bufs=1))
        x2_pool = ctx.enter_context(tc.tile_pool(name="x2", bufs=1))
        karena = ctx.enter_context(tc.tile_pool(name="karena", bufs=1))
        kx_pool = ctx.enter_context(tc.tile_pool(name="kx", bufs=1))
        q_pool = ctx.enter_context(tc.tile_pool(name="q", bufs=1))
        v_pool = ctx.enter_context(tc.tile_pool(name="v", bufs=1))
        wqa_pool = ctx.enter_context(tc.tile_pool(name="wqa", bufs=1))
        sln_pool = ctx.enter_context(tc.tile_pool(name="sln", bufs=1))
        dram = ctx.enter_context(tc.tile_pool(name="dram", bufs=1,
                                              space="DRAM"))

        bias_sb = const_pool.tile([P, 64], f32, tag="bias", name="bias")
        nc.sync.dma_start(bias_sb[:], bias_d[:])
        ones_bf = const_pool.tile([P, 1], bf16, tag="ones_bf", name="ones_bf")
        nc.vector.memset(ones_bf[:], 1.0)
        ones_row = const_pool.tile([1, P], bf16, tag="ones_row",
                                   name="ones_row")
        nc.vector.memset(ones_row[:], 1.0)

        def bcol(base, i):
            return bias_sb[:, base + i:base + i + 1]

        # gathered K/V arenas + own-q
        q_sb = [q_pool.tile([P, TQ], bf16, tag=f"q{i}", name=f"q{i}")
                for i in range(NCC)]
        k_sb = [kx_pool.tile([P, T], bf16, tag=f"kx{i}", name=f"k{i}")
                for i in range(NCC)]
        v_sb = [v_pool.tile([P, 16 * 65], bf16, tag=f"v{i}", name=f"v{i}")
                for i in range(NKC)]

        # AllGather bounce buffers (collectives are HBM<->HBM)
        k_in = dram.tile([D_MODEL, TQ], bf16, tag="kin", name="k_in")
        k_out = dram.tile([GROUPS * D_MODEL, TQ], bf16, tag="kout",
                          name="k_out")
        v_in = dram.tile([NTK * P, 16 * 65], bf16, tag="vin", name="v_in")
        v_out = dram.tile([NKC * P, 16 * 65], bf16, tag="vout",
                          name="v_out")

        # own x (bf16 feature-major); first in the DMA queue (gates LN1)
        with tc.tile_pool(name="xarena", bufs=1) as xarena:
            xb = []
            for ci in range(NCC):
                xt = xarena.tile([P, TQ], bf16, tag=f"x{ci}", name=f"x{ci}")
                nc.sync.dma_start(xt[:], x_d[ci * P:(ci + 1) * P, :])
                xb.append(xt)

            # prefetch K weights first (K starts the AllGather chain)
            wk = []
            for ci in range(NCC):
                wt = wqa_pool.tile([P, D_MODEL], bf16, tag=f"wq{ci}",
                                   name=f"wk{ci}")
                nc.sync.dma_start(wt[:], qkvw_d[ci * P:(ci + 1) * P,
                                                D_MODEL:2 * D_MODEL])
                wk.append(wt)

            # ---- LN1 for own tokens -> xln = (x - mu) * rsqrt(var + eps)
            xln = [xarena.tile([P, TQ], bf16, tag=f"xl{ci}", name=f"xln{ci}")
                   for ci in range(NCC)]
            with tc.tile_pool(name="lnt", bufs=1) as lnt, \
                 tc.tile_pool(name="xsq", bufs=2) as xsq_pool, \
                 tc.tile_pool(name="lnps", bufs=1, space="PSUM") as lnps, \
                 tc.tile_pool(name="lnbc", bufs=1, space="PSUM") as lnbc:
                st = lnps.tile([33, TQ], f32, tag="st", name="st")
                for ci in range(NCC):
                    xsq = xsq_pool.tile([P, TQ], bf16, tag="xsq", name="xsq")
                    nc.vector.tensor_mul(xsq[:], xb[ci][:], xb[ci][:])
                    nc.tensor.matmul(st[0:1, :], ones_bf[:], xb[ci][:],
                                     start=(ci == 0), stop=(ci == NCC - 1))
                    nc.tensor.matmul(st[32:33, :], ones_bf[:], xsq[:],
                                     start=(ci == 0), stop=(ci == NCC - 1))
                inv_n = 1.0 / D_MODEL
                mu = lnt.tile([1, TQ], f32, tag="mu", name="mu")
                nc.vector.tensor_scalar_mul(mu[:], st[0:1, :], inv_n)
                musq = lnt.tile([1, TQ], f32, tag="musq", name="musq")
                nc.vector.tensor_mul(musq[:], mu[:], mu[:])
                vpe = lnt.tile([1, TQ], f32, tag="vpe", name="vpe")
                nc.vector.tensor_scalar(vpe[:], st[32:33, :], inv_n, 1e-5,
                                        OP.mult, OP.add)
                nc.vector.tensor_sub(vpe[:], vpe[:], musq[:])
                rv = lnt.tile([1, TQ], f32, tag="rv", name="rv")
                nc.vector.reciprocal_approx_fast(rv[:], vpe[:])
                s_f = lnt.tile([1, TQ], f32, tag="sf", name="s_f")
                nc.scalar.sqrt(s_f[:], rv[:])
                s_bf = lnt.tile([1, TQ], bf16, tag="sbf", name="s_bf")
                nmu_bf = lnt.tile([1, TQ], bf16, tag="nmb", name="nmu_bf")
                with nc.allow_low_precision(reason="bf16 LN rows"):
                    nc.vector.tensor_copy(s_bf[:], s_f[:])
                    nc.vector.tensor_scalar_mul(nmu_bf[:], mu[:], -1.0)
                nm_bc = lnbc.tile([P, TQ], f32, tag="nmbc", name="nm_bc")
                nc.tensor.matmul(nm_bc[:], ones_row[:], nmu_bf[:])
                s_bc = lnbc.tile([P, TQ], f32, tag="sbc", name="s_bc")
                nc.tensor.matmul(s_bc[:], ones_row[:], s_bf[:])
                with tc.tile_pool(name="xct", bufs=2) as xct_pool:
                    for ci in range(NCC):
                        xc = xct_pool.tile([P, TQ], bf16, tag="xc", name="xc")
                        nc.vector.tensor_add(xc[:], xb[ci][:], nm_bc[:])
                        nc.vector.tensor_mul(xln[ci][:], xc[:], s_bc[:])

            # ---------------- own-token K / V / Q projections ----------
            with tc.tile_pool(name="kown", bufs=1) as kown_pool, \
                 tc.tile_pool(name="vown", bufs=1) as vown_pool, \
                 tc.tile_pool(name="qkvps", bufs=6, space="PSUM") as qkv_ps:

                # K own: K = Wk'.xln (bias dropped: cancels in softmax)
                k_own = []
                for co in range(NCC):
                    ps = qkv_ps.tile([P, TQ], f32, tag="ps", name="qkv_ps")
                    for ci in range(NCC):
                        nc.tensor.matmul(ps[:],
                                         wk[ci][:, co * P:(co + 1) * P],
                                         xln[ci][:], start=(ci == 0),
                                         stop=(ci == NCC - 1))
                    kt = kown_pool.tile([P, TQ], bf16, tag=f"ko{co}",
                                        name=f"k_own{co}")
                    nc.scalar.copy(kt[:], ps[:])
                    nc.sync.dma_start(k_in[co * P:(co + 1) * P, :], kt[:])
                    k_own.append(kt)
                nc.gpsimd.collective_compute(
                    "AllGather", OP.bypass, replica_groups=RG,
                    ins=[k_in.opt()], outs=[k_out.opt()])

                # V own, token-major with the ones column appended per head
                wv = []
                for ci in range(NCC):
                    wt = wqa_pool.tile([P, D_MODEL], bf16, tag=f"wq{ci}",
                                       name=f"wv{ci}")
                    nc.sync.dma_start(wt[:], qkvw_d[ci * P:(ci + 1) * P,
                                                    2 * D_MODEL:3 * D_MODEL])
                    wv.append(wt)
                v_own = [vown_pool.tile([P, 16 * 65], bf16, tag=f"vo{tk}",
                                        name=f"v_own{tk}")
                         for tk in range(NTK)]
                v3o = [v.rearrange("p (h s) -> p h s", s=65) for v in v_own]
                for tk in range(NTK):
                    nc.vector.memset(v3o[tk][:, :, 64:65], 1.0)
                    tsl = slice(tk * P, (tk + 1) * P)
                    for vh in range(2):
                        ps = qkv_ps.tile([P, TQ], f32, tag="ps",
                                         name="qkv_ps")
                        for ci in range(NCC):
                            nc.tensor.matmul(
                                ps[:], xln[ci][:, tsl],
                                wv[ci][:, vh * 512:(vh + 1) * 512],
                                start=(ci == 0), stop=(ci == NCC - 1))
                        src = ps.rearrange("p (h d) -> p h d", d=64)
                        nc.scalar.copy(v3o[tk][:, vh * 8:(vh + 1) * 8, 0:64],
                                       src[:])
                    nc.sync.dma_start(v_in[tk * P:(tk + 1) * P, :],
                                      v_own[tk][:])
                nc.gpsimd.collective_compute(
                    "AllGather", OP.bypass, replica_groups=RG,
                    ins=[v_in.opt()], outs=[v_out.opt()])

                # Q own (weights prefetched into the dead wk slots)
                wq = []
                for ci in range(NCC):
                    wt = wqa_pool.tile([P, D_MODEL], bf16, tag=f"wq{ci}",
                                       name=f"wq{ci}")
                    nc.sync.dma_start(wt[:],
                                      qkvw_d[ci * P:(ci + 1) * P, 0:D_MODEL])
                    wq.append(wt)
                for co in range(NCC):
                    ps = qkv_ps.tile([P, TQ], f32, tag="ps", name="qkv_ps")
                    for ci in range(NCC):
                        nc.tensor.matmul(ps[:],
                                         wq[ci][:, co * P:(co + 1) * P],
                                         xln[ci][:], start=(ci == 0),
                                         stop=(ci == NCC - 1))
                    nc.scalar.activation(q_sb[co][:], ps[:], AF.Identity,
                                         bias=bcol(QB, co))

                # gather the grouped K/V back into SBUF (global key order)
                for r in range(GROUPS):
                    for co in range(NCC):
                        nc.sync.dma_start(
                            k_sb[co][:, r * TQ:(r + 1) * TQ],
                            k_out[r * D_MODEL + co * P:
                                  r * D_MODEL + (co + 1) * P, :])
                for r in range(GROUPS):
                    for tk in range(NTK):
                        nc.sync.dma_start(
                            v_sb[r * NTK + tk][:],
                            v_out[(r * NTK + tk) * P:
                                  (r * NTK + tk + 1) * P, :])

        # ---------------- attention + output projection ----------------
        # Pipelined across heads: head h's scores/exp stream while head
        # h-1's PV accumulates. The softmax normalize is per-pair: the
        # denominator reciprocals run on DVE off the critical path, then
        # two tiny broadcast matmuls + one DVE mul per head pair.
        x2 = [x2_pool.tile([P, TQ], f32, tag=f"x2{i}", name=f"x2_{i}")
              for i in range(NCC)]
        xbc = [q_pool.tile([P, TQ], bf16, tag=f"q{i}", name=f"xb2c{i}")
               for i in range(NCC)]

        with tc.tile_pool(name="attn", bufs=1) as attn_pool:
            attn_sb = [attn_pool.tile([P, TQ], bf16, tag=f"a{i}",
                                      name=f"attn{i}") for i in range(NCC)]
            # o_w prefetch into the dead wq slots; lands early in attention
            ow = []
            for ci in range(NCC):
                wt = wqa_pool.tile([P, D_MODEL], bf16, tag=f"wq{ci}",
                                   name=f"ow{ci}")
                nc.sync.dma_start(wt[:], ow_d[ci * P:(ci + 1) * P, :])
                ow.append(wt)

            with tc.tile_pool(name="es", bufs=12) as es_pool, \
                 tc.tile_pool(name="scps", bufs=2, space="PSUM") as sc_ps, \
                 tc.tile_pool(name="pvps", bufs=2, space="PSUM") as pv_psp, \
                 tc.tile_pool(name="rbps", bufs=2, space="PSUM") as rb_psp, \
                 tc.tile_pool(name="rcp", bufs=2) as rcp_pool:

                rcf_cur = [None]

                def pv_finish(hd, pv):
                    # drain + denominator recip; normalize per finished pair
                    ct, ro = hd // 2, (hd % 2) * 64
                    ro8 = (hd % 2) * TQ
                    with nc.allow_low_precision(reason="unnorm bf16 PV"):
                        nc.vector.tensor_copy(attn_sb[ct][ro:ro + 64, :],
                                              pv[0:64, :])
                    if hd % 2 == 0:
                        rcf_cur[0] = rcp_pool.tile([1, 2 * TQ], f32,
                                                   tag="rcf", name="recf")
                    nc.vector.reciprocal(rcf_cur[0][:, ro8:ro8 + TQ],
                                         pv[64:65, :])
                    if hd % 2 == 1:
                        recb = rcp_pool.tile([1, 2 * TQ], bf16, tag="rcb",
                                             name="recb")
                        with nc.allow_low_precision(reason="bf16 recip rows"):
                            nc.vector.tensor_copy(recb[:], rcf_cur[0][:])
                        rb = rb_psp.tile([P, TQ], f32, tag="rb", name="rb_ps")
                        nc.tensor.matmul(rb[0:64, :], ones_row[:, 0:64],
                                         recb[:, 0:TQ])
                        nc.tensor.matmul(rb[64:P, :], ones_row[:, 0:64],
                                         recb[:, TQ:2 * TQ])
                        nc.vector.tensor_mul(attn_sb[ct][:], attn_sb[ct][:],
                                             rb[:])

                def sc_pair(hd, tp, ksl, qsl, es_cur):
                    ps = sc_ps.tile([P, 2 * TQ], f32, tag="sc", name="sc_ps")
                    nc.tensor.matmul(ps[:, 0:TQ],
                                     ksl[:, (2 * tp) * P:(2 * tp + 1) * P],
                                     qsl)
                    nc.tensor.matmul(ps[:, TQ:2 * TQ],
                                     ksl[:, (2 * tp + 1) * P:
                                         (2 * tp + 2) * P], qsl)
                    e = es_pool.tile([P, 2 * TQ], bf16, tag="es", name="es")
                    nc.scalar.activation(e[:], ps[:], AF.Exp,
                                         scale=1.0 / np.sqrt(HEAD_DIM))
                    es_cur.append(e)

                es_prev, es_cur = None, None
                for hd in range(N_HEAD + 1):
                    es_prev, es_cur = es_cur, []
                    if es_prev is not None:
                        pv_prev = pv_psp.tile([65, TQ], f32, tag="pv",
                                              name="pv_ps")
                        for tk in range(NKC):
                            nc.tensor.matmul(
                                pv_prev[:],
                                v_sb[tk][:, (hd - 1) * 65:hd * 65],
                                es_prev[tk // 2][:, (tk % 2) * TQ:
                                                 (tk % 2 + 1) * TQ],
                                start=(tk == 0), stop=(tk == NKC - 1))
                    if hd < N_HEAD:
                        ct, ro = hd // 2, (hd % 2) * 64
                        ksl = k_sb[ct][ro:ro + 64, :]
                        qsl = q_sb[ct][ro:ro + 64, :]
                        for tp in range(NKC // 2):
                            sc_pair(hd, tp, ksl, qsl, es_cur)
                    if es_prev is not None:
                        pv_finish(hd - 1, pv_prev)

            # xo borrows the v slots (v is dead after the last PV matmul)
            xo = [v_pool.tile([P, TQ], f32, tag=f"v{i}", name=f"xo{i}")
                  for i in range(NCC)]
            for ci in range(NCC):
                nc.sync.dma_start(xo[ci][:], xo_d[ci * P:(ci + 1) * P, :])

            # FFN1 first-half weights: allocate into the dead K slots now so
            # the DMAs run during the normalize/O-proj tail (per-slot WAR on
            # the last scores read; 512-col chunks land progressively)
            w1t0 = []
            for ci in range(NCC):
                wt = kx_pool.tile([P, 2048], bf16, tag=f"kx{ci}",
                                  name=f"w1t{ci}p0")
                for qc in range(4):
                    nc.sync.dma_start(
                        wt[:, qc * 512:(qc + 1) * 512],
                        w1_d[ci * P:(ci + 1) * P, qc * 512:(qc + 1) * 512])
                w1t0.append(wt)

            with tc.tile_pool(name="ops", bufs=4, space="PSUM") as o_ps:
                for co in range(NCC):
                    ps = o_ps.tile([P, TQ], f32, tag="ps", name="o_ps")
                    for hi in range(NCC):
                        nc.tensor.matmul(ps[:], ow[hi][:, co * P:(co + 1) * P],
                                         attn_sb[hi][:], start=(hi == 0),
                                         stop=(hi == NCC - 1))
                    nc.vector.scalar_tensor_tensor(x2[co][:], ps[:],
                                                   bcol(OB, co), xo[co][:],
                                                   OP.add, OP.add)

        # ------- LN2 over x2; fold into xbc = (x2 - mu2) * s2 (bf16)
        with tc.tile_pool(name="xq2", bufs=2) as xqp, \
             tc.tile_pool(name="ln2ps", bufs=1, space="PSUM") as ln2ps, \
             tc.tile_pool(name="ln2bc", bufs=1, space="PSUM") as ln2bc, \
             tc.tile_pool(name="ln2t", bufs=2) as ln2t:
            st2 = ln2ps.tile([33, TQ], f32, tag="st2", name="st2")
            xc2 = []
            for ci in range(NCC):
                xc = xqp.tile([P, TQ], bf16, tag=f"xc{ci}", name=f"xc2_{ci}")
                nc.scalar.copy(xc[:], x2[ci][:])
                xq = xqp.tile([P, TQ], bf16, tag="xq", name="xq2", bufs=2)
                nc.vector.tensor_mul(xq[:], xc[:], xc[:])
                nc.tensor.matmul(st2[0:1, :], ones_bf[:], xc[:],
                                 start=(ci == 0), stop=(ci == NCC - 1))
                nc.tensor.matmul(st2[32:33, :], ones_bf[:], xq[:],
                                 start=(ci == 0), stop=(ci == NCC - 1))
                xc2.append(xc)
            inv_n = 1.0 / D_MODEL
            mu2_sb = ln2t.tile([1, TQ], f32, tag="mu", name="mu2_sb")
            nc.vector.tensor_scalar_mul(mu2_sb[:], st2[0:1, :], inv_n)
            mu2sq = ln2t.tile([1, TQ], f32, tag="musq", name="mu2sq")
            nc.vector.tensor_mul(mu2sq[:], mu2_sb[:], mu2_sb[:])
            vpe = ln2t.tile([1, TQ], f32, tag="vpe", name="vpe2")
            nc.vector.tensor_scalar(vpe[:], st2[32:33, :], inv_n, 1e-5,
                                    OP.mult, OP.add)
            nc.vector.tensor_sub(vpe[:], vpe[:], mu2sq[:])
            rv = ln2t.tile([1, TQ], f32, tag="rv", name="rv2")
            nc.vector.reciprocal_approx_fast(rv[:], vpe[:])
            s2_f = ln2t.tile([1, TQ], f32, tag="ri", name="s2_f")
            nc.scalar.sqrt(s2_f[:], rv[:])
            s2_bf = ln2t.tile([1, TQ], bf16, tag="sbf", name="s2_bf")
            nmu2_bf = ln2t.tile([1, TQ], bf16, tag="nmb", name="nmu2_bf")
            with nc.allow_low_precision(reason="bf16 LN2 rows"):
                nc.vector.tensor_copy(s2_bf[:], s2_f[:])
                nc.vector.tensor_scalar_mul(nmu2_bf[:], mu2_sb[:], -1.0)
            nm2_bc = ln2bc.tile([P, TQ], f32, tag="nmbc", name="nm2_bc")
            nc.tensor.matmul(nm2_bc[:], ones_row[:], nmu2_bf[:])
            s2_bc = ln2bc.tile([P, TQ], f32, tag="sbc", name="s2_bc")
            nc.tensor.matmul(s2_bc[:], ones_row[:], s2_bf[:])
            with tc.tile_pool(name="xct2", bufs=2) as xct2_pool:
                for ci in range(NCC):
                    xc = xct2_pool.tile([P, TQ], bf16, tag="xc", name="xc2")
                    nc.vector.tensor_add(xc[:], xc2[ci][:], nm2_bc[:])
                    nc.vector.tensor_mul(xbc[ci][:], xc[:], s2_bc[:])

        # ---------------- FFN ----------------
        # h1 [4096, 512] lives in the K-arena slots as 8 groups of 4 f-chunks
        hg = [karena.tile([P, T], bf16, tag=f"k{i}", name=f"hg{i}")
              for i in range(NCC)]

        def h1sl(fch):
            return hg[fch // 4][:, (fch % 4) * 512:(fch % 4 + 1) * 512]

        with tc.tile_pool(name="h1ps", bufs=4, space="PSUM") as h1_ps:
            for fp in range(2):
                if fp == 0:
                    w1t = w1t0
                else:
                    w1t = []
                    for ci in range(NCC):
                        wt = kx_pool.tile([P, 2048], bf16, tag=f"kx{ci}",
                                          name=f"w1t{ci}p{fp}")
                        for qc in range(4):
                            nc.sync.dma_start(
                                wt[:, qc * 512:(qc + 1) * 512],
                                w1_d[ci * P:(ci + 1) * P,
                                     fp * 2048 + qc * 512:
                                     fp * 2048 + (qc + 1) * 512])
                        w1t.append(wt)
                for fo in range(16):
                    fch = fp * 16 + fo
                    ps = h1_ps.tile([P, TQ], f32, tag="ps", name="h1_ps")
                    for ci in range(NCC):
                        nc.tensor.matmul(ps[:],
                                         w1t[ci][:, fo * P:(fo + 1) * P],
                                         xbc[ci][:], start=(ci == 0),
                                         stop=(ci == NCC - 1))
                    nc.scalar.activation(h1sl(fch), ps[:], AF.Gelu,
                                         bias=bcol(B1, fch))

        with tc.tile_pool(name="outps", bufs=1, space="PSUM") as out_ps, \
             tc.tile_pool(name="outsb", bufs=1) as out_pool:
            ops = [out_ps.tile([P, TQ], f32, tag=f"o{co}", name=f"out_ps{co}")
                   for co in range(NCC)]
            for fch in range(NFC):
                wt = wqa_pool.tile([P, D_MODEL], bf16, tag=f"wq{fch % 8}",
                                   name=f"w2t{fch}")
                nc.sync.dma_start(wt[:], w2_d[fch * P:(fch + 1) * P, :])
                for co in range(NCC):
                    nc.tensor.matmul(ops[co][:], wt[:, co * P:(co + 1) * P],
                                     h1sl(fch),
                                     start=(fch == 0), stop=(fch == NFC - 1))
            for co in range(NCC):
                osb = out_pool.tile([P, TQ], f32, tag=f"os{co}",
                                    name=f"osb{co}")
                nc.vector.scalar_tensor_tensor(osb[:], ops[co][:],
                                               bcol(B2, co), x2[co][:],
                                               OP.add, OP.add)
                nc.sync.dma_start(out_d[co * P:(co + 1) * P, :], osb[:])


def _prep_inputs(x, qkv_w, qkv_b, o_w, o_b, ln1_g, ln1_b,
                 ffn_w1, ffn_b1, ffn_w2, ffn_b2, ln2_g, ln2_b):
    import ml_dtypes
    bf = ml_dtypes.bfloat16
    f8 = np.float64

    # fold LN gammas into the following projection weights, LN betas and
    # projection biases into per-output-feature constants (data-independent)
    Wg = qkv_w.astype(f8) * ln1_g.astype(f8)[None, :]
    cvec = qkv_w.astype(f8) @ ln1_b.astype(f8) + qkv_b.astype(f8)
    qkv_wT = np.ascontiguousarray(Wg.T.astype(np.float32)).astype(bf)
    ob_eff = (o_b.astype(f8) + o_w.astype(f8) @ cvec[2 * D_MODEL:]
              ).astype(np.float32)

    W1g = ffn_w1.astype(f8) * ln2_g.astype(f8)[None, :]
    b1_eff = (ffn_w1.astype(f8) @ ln2_b.astype(f8)
              + ffn_b1.astype(f8)).astype(np.float32)
    w1T = np.ascontiguousarray(W1g.T.astype(np.float32)).astype(bf)

    o_wT = np.ascontiguousarray(o_w.T).astype(bf)
    w2T = np.ascontiguousarray(ffn_w2.T).astype(bf)

    def cols(v, n):
        return np.ascontiguousarray(v.reshape(n, P).T.astype(np.float32))

    biases = np.zeros((P, 64), np.float32)
    biases[:, QB:QB + 8] = cols(cvec[0:D_MODEL].astype(np.float32), 8)
    biases[:, OB:OB + 8] = cols(ob_eff, 8)
    biases[:, B1:B1 + 32] = cols(b1_eff, 32)
    biases[:, B2:B2 + 8] = cols(ffn_b2, 8)

    in_maps = []
    for c in range(N_CORES):
        b, s = c // GROUPS, c % GROUPS
        xs = np.ascontiguousarray(x[b][s * TQ:(s + 1) * TQ, :].T)
        in_maps.append({
            "x_fm": xs.astype(bf),
            "x_own": xs.astype(np.float32),
            "qkv_wT": qkv_wT,
            "o_wT": o_wT,
            "w1T": w1T,
            "w2T": w2T,
            "biases": biases,
        })
    return in_maps


def kernel(**inputs):
    from concourse.bass_utils import run_bass_kernel_spmd

    if "nc" not in _cache:
        _cache["nc"] = _build()
    nc = _cache["nc"]

    inputs = {k: np.asarray(v, dtype=np.float32) for k, v in inputs.items()}
    in_maps = _prep_inputs(**inputs)

    res = run_bass_kernel_spmd(nc, in_maps, core_ids=list(range(N_CORES)),
                               **_cache.get("run_kwargs", {}))
    _cache["last_results"] = res

    out = np.empty((B, T, D_MODEL), np.float32)
    for c in range(N_CORES):
        b, s = c // GROUPS, c % GROUPS
        out[b, s * TQ:(s + 1) * TQ, :] = res.results[c]["out"].T
    return out


# revision 21
# speedup vs baseline: 1.0678x; 1.0364x over previous
"""Trainium2 Bass kernel for a minimal transformer block (B=2, T=2048, C=1024,
H=16, Dh=64, F=4096), sharded over 8 NeuronCores.

Sharding: data-parallel over batch (2 groups of 4 cores) x sequence-parallel
over tokens within each batch (512 tokens per core). Each core computes
Q/K/V only for its own 512 tokens from a pre-normalized activation
xln = (x - mu) * rsqrt(var + eps) (LN gammas/betas folded into the weights /
bias table on the host); K and V are then AllGathered across the 4 cores of
each batch group so every core attends over the full 2048 keys. Keys are kept
in global token order (softmax without a mask is permutation-invariant).

Everything on-chip is feature-major ([features, tokens]); the host transposes
inputs/outputs and pre-transposes/casts weights to bf16.
"""

import sys

if "/opt/trn_rl_repo" not in sys.path:
    sys.path.insert(0, "/opt/trn_rl_repo")

import numpy as np

D_MODEL = 1024
N_HEAD = 16
HEAD_DIM = 64
D_FF = 4096
B = 2
T = 2048
N_CORES = 8
GROUPS = 4          # cores per batch
TQ = T // GROUPS    # own tokens per core = 512
P = 128
NCC = D_MODEL // P  # 8 C-chunks
NKC = T // P        # 16 k-chunks of 128
NTK = TQ // P       # 4 own-token chunks of 128
NFC = D_FF // P     # 32 f-chunks of 128

# bias-table column layout ([128, 64] f32)
QB, KB, OB, B1, B2 = 0, 8, 16, 24, 56

_cache = {}


def _build():
    import concourse.bass as bass
    import concourse.tile as tile
    from concourse import bacc, mybir

    f32 = mybir.dt.float32
    bf16 = mybir.dt.bfloat16
    AF = mybir.ActivationFunctionType
    OP = mybir.AluOpType

    nc = bacc.Bacc("TRN2", target_bir_lowering=False, debug=False,
                   num_devices=N_CORES)

    x_d = nc.dram_tensor("x_fm", [D_MODEL, TQ], bf16,
                         kind="ExternalInput").ap()
    xo_d = nc.dram_tensor("x_own", [D_MODEL, TQ], f32,
                          kind="ExternalInput").ap()
    qkvw_d = nc.dram_tensor("qkv_wT", [D_MODEL, 3 * D_MODEL], bf16,
                            kind="ExternalInput").ap()
    ow_d = nc.dram_tensor("o_wT", [D_MODEL, D_MODEL], bf16,
                          kind="ExternalInput").ap()
    w1_d = nc.dram_tensor("w1T", [D_MODEL, D_FF], bf16,
                          kind="ExternalInput").ap()
    w2_d = nc.dram_tensor("w2T", [D_FF, D_MODEL], bf16,
                          kind="ExternalInput").ap()
    bias_d = nc.dram_tensor("biases", [P, 64], f32, kind="ExternalInput").ap()
    wbark_d = nc.dram_tensor("wbar_k", [1, D_MODEL], bf16,
                             kind="ExternalInput").ap()
    out_d = nc.dram_tensor("out", [D_MODEL, TQ], f32,
                           kind="ExternalOutput").ap()

    with tile.TileContext(nc) as tc:
        _body(tc, bass, mybir, f32, bf16, AF, OP, x_d, xo_d, qkvw_d,
              ow_d, w1_d, w2_d, bias_d, wbark_d, out_d)

    nc.compile()
    return nc


def _body(tc, bass, mybir, f32, bf16, AF, OP, x_d, xo_d, qkvw_d,
          ow_d, w1_d, w2_d, bias_d, wbark_d, out_d):
    nc = tc.nc
    from contextlib import ExitStack

    RG = [[0, 1, 2, 3], [4, 5, 6, 7]]

    ctx = ExitStack()
    with ctx:
        # ---- persistent arena (stack-allocated; open for the whole kernel)
        const_pool = ctx.enter_context(tc.tile_pool(name="const", bufs=1))
        x2_pool = ctx.enter_context(tc.tile_pool(name="x2", bufs=1))
        karena = ctx.enter_context(tc.tile_pool(name="karena", bufs=1))
        kx_pool = ctx.enter_context(tc.tile_pool(name="kx", bufs=1))
        q_pool = ctx.enter_context(tc.tile_pool(name="q", bufs=1))
        v_pool = ctx.enter_context(tc.tile_pool(name="v", bufs=1))
        wqa_pool = ctx.enter_context(tc.tile_pool(name="wqa", bufs=1))
        sln_pool = ctx.enter_context(tc.tile_pool(name="sln", bufs=1))
        dram = ctx.enter_context(tc.tile_pool(name="dram", bufs=1,
                                              space="DRAM"))

        bias_sb = const_pool.tile([P, 64], f32, tag="bias", name="bias")
        nc.sync.dma_start(bias_sb[:], bias_d[:])
        ones_bf = const_pool.tile([P, 1], bf16, tag="ones_bf", name="ones_bf")
        nc.vector.memset(ones_bf[:], 1.0)
        ones_row = const_pool.tile([1, P], bf16, tag="ones_row",
                                   name="ones_row")
        nc.vector.memset(ones_row[:], 1.0)
        wbark_sb = const_pool.tile([1, D_MODEL], bf16, tag="wbk",
                                   name="wbark_sb")
        nc.sync.dma_start(wbark_sb[:], wbark_d[:])

        def bcol(base, i):
            return bias_sb[:, base + i:base + i + 1]

        # gathered K/V arenas + own-q
        q_sb = [q_pool.tile([P, TQ], bf16, tag=f"q{i}", name=f"q{i}")
                for i in range(NCC)]
        k_sb = [kx_pool.tile([P, T], bf16, tag=f"kx{i}", name=f"k{i}")
                for i in range(NCC)]
        v_sb = [v_pool.tile([P, 16 * 65], bf16, tag=f"v{i}", name=f"v{i}")
                for i in range(NKC)]

        # AllGather bounce buffers (collectives are HBM<->HBM)
        k_in = dram.tile([D_MODEL, TQ], bf16, tag="kin", name="k_in")
        k_out = dram.tile([GROUPS * D_MODEL, TQ], bf16, tag="kout",
                          name="k_out")
        v_in = dram.tile([NTK * P, 16 * 65], bf16, tag="vin", name="v_in")
        v_out = dram.tile([NKC * P, 16 * 65], bf16, tag="vout",
                          name="v_out")

        # own x (bf16 feature-major); first in the DMA queue (gates LN1)
        with tc.tile_pool(name="xarena", bufs=1) as xarena:
            xb = []
            for ci in range(NCC):
                xt = xarena.tile([P, TQ], bf16, tag=f"x{ci}", name=f"x{ci}")
                nc.sync.dma_start(xt[:], x_d[ci * P:(ci + 1) * P, :])
                xb.append(xt)

            # prefetch K weights first (K starts the AllGather chain)
            wk = []
            for ci in range(NCC):
                wt = wqa_pool.tile([P, D_MODEL], bf16, tag=f"wq{ci}",
                                   name=f"wk{ci}")
                nc.sync.dma_start(wt[:], qkvw_d[ci * P:(ci + 1) * P,
                                                D_MODEL:2 * D_MODEL])
                wk.append(wt)

            # ---- LN1 for own tokens -> xln = (x - mu) * rsqrt(var + eps)
            xln = [xarena.tile([P, TQ], bf16, tag=f"xl{ci}", name=f"xln{ci}")
                   for ci in range(NCC)]
            with tc.tile_pool(name="kown", bufs=1) as kown_pool, \
                 tc.tile_pool(name="lnt", bufs=1) as lnt, \
                 tc.tile_pool(name="xsq", bufs=2) as xsq_pool, \
                 tc.tile_pool(name="lnps", bufs=1, space="PSUM") as lnps, \
                 tc.tile_pool(name="lnbc", bufs=1, space="PSUM") as lnbc:
                st = lnps.tile([33, TQ], f32, tag="st", name="st")
                for ci in range(NCC):
                    xsq = xsq_pool.tile([P, TQ], bf16, tag="xsq", name="xsq")
                    nc.vector.tensor_mul(xsq[:], xb[ci][:], xb[ci][:])
                    nc.tensor.matmul(st[0:1, :], ones_bf[:], xb[ci][:],
                                     start=(ci == 0), stop=(ci == NCC - 1))
                    nc.tensor.matmul(st[32:33, :], ones_bf[:], xsq[:],
                                     start=(ci == 0), stop=(ci == NCC - 1))
                inv_n = 1.0 / D_MODEL
                mu = lnt.tile([1, TQ], f32, tag="mu", name="mu")
                nc.vector.tensor_scalar_mul(mu[:], st[0:1, :], inv_n)
                musq = lnt.tile([1, TQ], f32, tag="musq", name="musq")
                nc.vector.tensor_mul(musq[:], mu[:], mu[:])
                vpe = lnt.tile([1, TQ], f32, tag="vpe", name="vpe")
                nc.vector.tensor_scalar(vpe[:], st[32:33, :], inv_n, 1e-5,
                                        OP.mult, OP.add)
                nc.vector.tensor_sub(vpe[:], vpe[:], musq[:])
                rv = lnt.tile([1, TQ], f32, tag="rv", name="rv")
                nc.vector.reciprocal_approx_fast(rv[:], vpe[:])
                s_f = lnt.tile([1, TQ], f32, tag="sf", name="s_f")
                nc.scalar.sqrt(s_f[:], rv[:])
                s_bf = lnt.tile([1, TQ], bf16, tag="sbf", name="s_bf")
                nmu_bf = lnt.tile([1, TQ], bf16, tag="nmb", name="nmu_bf")
                with nc.allow_low_precision(reason="bf16 LN rows"):
                    nc.vector.tensor_copy(s_bf[:], s_f[:])
                    nc.vector.tensor_scalar_mul(nmu_bf[:], mu[:], -1.0)
                # K own from raw x with the rank-1 mean correction: the
                # x-matmuls start before the LN rows are even done, so the
                # K AllGather (the latency-critical one) kicks earliest:
                #   K = s_t * (Wk'.x_t + wbar_k.(-mu_t))
                nm_bc = lnbc.tile([P, TQ], f32, tag="nmbc", name="nm_bc")
                nc.tensor.matmul(nm_bc[:], ones_row[:], nmu_bf[:])
                s_bc = lnbc.tile([P, TQ], f32, tag="sbc", name="s_bc")
                nc.tensor.matmul(s_bc[:], ones_row[:], s_bf[:])
                s_bb = lnt.tile([P, TQ], bf16, tag="sbb", name="s_bb")
                nc.scalar.copy(s_bb[:], s_bc[:])
                with tc.tile_pool(name="kps", bufs=3,
                                  space="PSUM") as kps_pool:
                    k_own = []
                    for co in range(NCC):
                        ps = kps_pool.tile([P, TQ], f32, tag="ps",
                                           name="k_ps")
                        for ci in range(NCC):
                            nc.tensor.matmul(ps[:],
                                             wk[ci][:, co * P:(co + 1) * P],
                                             xb[ci][:], start=(ci == 0),
                                             stop=False)
                        nc.tensor.matmul(ps[:],
                                         wbark_sb[:, co * P:(co + 1) * P],
                                         nmu_bf[:], start=False, stop=True)
                        kt = kown_pool.tile([P, TQ], bf16, tag=f"ko{co}",
                                            name=f"k_own{co}")
                        nc.vector.tensor_mul(kt[:], ps[:], s_bb[:])
                        nc.sync.dma_start(k_in[co * P:(co + 1) * P, :],
                                          kt[:])
                        k_own.append(kt)
                    nc.gpsimd.collective_compute(
                        "AllGather", OP.bypass, replica_groups=RG,
                        ins=[k_in.opt()], outs=[k_out.opt()])
                with tc.tile_pool(name="xct", bufs=2) as xct_pool:
                    for ci in range(NCC):
                        xc = xct_pool.tile([P, TQ], bf16, tag="xc", name="xc")
                        nc.vector.tensor_add(xc[:], xb[ci][:], nm_bc[:])
                        nc.vector.tensor_mul(xln[ci][:], xc[:], s_bc[:])

            # ---------------- own-token V / Q projections ----------
            with tc.tile_pool(name="vown", bufs=1) as vown_pool, \
                 tc.tile_pool(name="qkvps", bufs=6, space="PSUM") as qkv_ps:

                # V own, token-major with the ones column appended per head
                wv = []
                for ci in range(NCC):
                    wt = wqa_pool.tile([P, D_MODEL], bf16, tag=f"wq{ci}",
                                       name=f"wv{ci}")
                    nc.sync.dma_start(wt[:], qkvw_d[ci * P:(ci + 1) * P,
                                                    2 * D_MODEL:3 * D_MODEL])
                    wv.append(wt)
                v_own = [vown_pool.tile([P, 16 * 65], bf16, tag=f"vo{tk}",
                                        name=f"v_own{tk}")
                         for tk in range(NTK)]
                v3o = [v.rearrange("p (h s) -> p h s", s=65) for v in v_own]
                for tk in range(NTK):
                    nc.vector.memset(v3o[tk][:, :, 64:65], 1.0)
                    tsl = slice(tk * P, (tk + 1) * P)
                    for vh in range(2):
                        ps = qkv_ps.tile([P, TQ], f32, tag="ps",
                                         name="qkv_ps")
                        for ci in range(NCC):
                            nc.tensor.matmul(
                                ps[:], xln[ci][:, tsl],
                                wv[ci][:, vh * 512:(vh + 1) * 512],
                                start=(ci == 0), stop=(ci == NCC - 1))
                        src = ps.rearrange("p (h d) -> p h d", d=64)
                        nc.scalar.copy(v3o[tk][:, vh * 8:(vh + 1) * 8, 0:64],
                                       src[:])
                    nc.sync.dma_start(v_in[tk * P:(tk + 1) * P, :],
                                      v_own[tk][:])
                nc.gpsimd.collective_compute(
                    "AllGather", OP.bypass, replica_groups=RG,
                    ins=[v_in.opt()], outs=[v_out.opt()])

                # Q own (weights prefetched into the dead wk slots)
                wq = []
                for ci in range(NCC):
                    wt = wqa_pool.tile([P, D_MODEL], bf16, tag=f"wq{ci}",
                                       name=f"wq{ci}")
                    nc.sync.dma_start(wt[:],
                                      qkvw_d[ci * P:(ci + 1) * P, 0:D_MODEL])
                    wq.append(wt)
                for co in range(NCC):
                    ps = qkv_ps.tile([P, TQ], f32, tag="ps", name="qkv_ps")
                    for ci in range(NCC):
                        nc.tensor.matmul(ps[:],
                                         wq[ci][:, co * P:(co + 1) * P],
                                         xln[ci][:], start=(ci == 0),
                                         stop=(ci == NCC - 1))
                    nc.scalar.activation(q_sb[co][:], ps[:], AF.Identity,
                                         bias=bcol(QB, co))

                # gather the grouped K/V back into SBUF (global key order;
                # co-major so the first heads' K lands first)
                for co in range(NCC):
                    for r in range(GROUPS):
                        nc.sync.dma_start(
                            k_sb[co][:, r * TQ:(r + 1) * TQ],
                            k_out[r * D_MODEL + co * P:
                                  r * D_MODEL + (co + 1) * P, :])
                for r in range(GROUPS):
                    for tk in range(NTK):
                        nc.sync.dma_start(
                            v_sb[r * NTK + tk][:],
                            v_out[(r * NTK + tk) * P:
                                  (r * NTK + tk + 1) * P, :])

        # ---------------- attention + output projection ----------------
        # Pipelined across heads: head h's scores/exp stream while head
        # h-1's PV accumulates. The softmax normalize is per-pair: the
        # denominator reciprocals run on DVE off the critical path, then
        # two tiny broadcast matmuls + one DVE mul per head pair.
        x2 = [x2_pool.tile([P, TQ], f32, tag=f"x2{i}", name=f"x2_{i}")
              for i in range(NCC)]
        xbc = [q_pool.tile([P, TQ], bf16, tag=f"q{i}", name=f"xb2c{i}")
               for i in range(NCC)]

        with tc.tile_pool(name="attn", bufs=1) as attn_pool:
            attn_sb = [attn_pool.tile([P, TQ], bf16, tag=f"a{i}",
                                      name=f"attn{i}") for i in range(NCC)]
            # o_w prefetch into the dead wq slots; lands early in attention
            ow = []
            for ci in range(NCC):
                wt = wqa_pool.tile([P, D_MODEL], bf16, tag=f"wq{ci}",
                                   name=f"ow{ci}")
                nc.sync.dma_start(wt[:], ow_d[ci * P:(ci + 1) * P, :])
                ow.append(wt)

            with tc.tile_pool(name="es", bufs=12) as es_pool, \
                 tc.tile_pool(name="scps", bufs=2, space="PSUM") as sc_ps, \
                 tc.tile_pool(name="pvps", bufs=2, space="PSUM") as pv_psp, \
                 tc.tile_pool(name="rbps", bufs=2, space="PSUM") as rb_psp, \
                 tc.tile_pool(name="rcp", bufs=2) as rcp_pool:

                rcf_cur = [None]

                def pv_finish(hd, pv):
                    # drain + denominator recip; normalize per finished pair
                    ct, ro = hd // 2, (hd % 2) * 64
                    ro8 = (hd % 2) * TQ
                    with nc.allow_low_precision(reason="unnorm bf16 PV"):
                        nc.vector.tensor_copy(attn_sb[ct][ro:ro + 64, :],
                                              pv[0:64, :])
                    if hd % 2 == 0:
                        rcf_cur[0] = rcp_pool.tile([1, 2 * TQ], f32,
                                                   tag="rcf", name="recf")
                    nc.vector.reciprocal(rcf_cur[0][:, ro8:ro8 + TQ],
                                         pv[64:65, :])
                    if hd % 2 == 1:
                        recb = rcp_pool.tile([1, 2 * TQ], bf16, tag="rcb",
                                             name="recb")
                        with nc.allow_low_precision(reason="bf16 recip rows"):
                            nc.vector.tensor_copy(recb[:], rcf_cur[0][:])
                        rb = rb_psp.tile([P, TQ], f32, tag="rb", name="rb_ps")
                        nc.tensor.matmul(rb[0:64, :], ones_row[:, 0:64],
                                         recb[:, 0:TQ])
                        nc.tensor.matmul(rb[64:P, :], ones_row[:, 0:64],
                                         recb[:, TQ:2 * TQ])
                        nc.vector.tensor_mul(attn_sb[ct][:], attn_sb[ct][:],
                                             rb[:])

                def sc_pair(hd, tp, ksl, qsl, es_cur):
                    ps = sc_ps.tile([P, 2 * TQ], f32, tag="sc", name="sc_ps")
                    nc.tensor.matmul(ps[:, 0:TQ],
                                     ksl[:, (2 * tp) * P:(2 * tp + 1) * P],
                                     qsl)
                    nc.tensor.matmul(ps[:, TQ:2 * TQ],
                                     ksl[:, (2 * tp + 1) * P:
                                         (2 * tp + 2) * P], qsl)
                    e = es_pool.tile([P, 2 * TQ], bf16, tag="es", name="es")
                    nc.scalar.activation(e[:], ps[:], AF.Exp,
                                         scale=1.0 / np.sqrt(HEAD_DIM))
                    es_cur.append(e)

                es_prev, es_cur = None, None
                for hd in range(N_HEAD + 1):
                    es_prev, es_cur = es_cur, []
                    if es_prev is not None:
                        pv_prev = pv_psp.tile([65, TQ], f32, tag="pv",
                                              name="pv_ps")
                        for tk in range(NKC):
                            nc.tensor.matmul(
                                pv_prev[:],
                                v_sb[tk][:, (hd - 1) * 65:hd * 65],
                                es_prev[tk // 2][:, (tk % 2) * TQ:
                                                 (tk % 2 + 1) * TQ],
                                start=(tk == 0), stop=(tk == NKC - 1))
                    if hd < N_HEAD:
                        ct, ro = hd // 2, (hd % 2) * 64
                        ksl = k_sb[ct][ro:ro + 64, :]
                        qsl = q_sb[ct][ro:ro + 64, :]
                        for tp in range(NKC // 2):
                            sc_pair(hd, tp, ksl, qsl, es_cur)
                    if es_prev is not None:
                        pv_finish(hd - 1, pv_prev)

            # xo borrows the v slots (v is dead after the last PV matmul)
            xo = [v_pool.tile([P, TQ], f32, tag=f"v{i}", name=f"xo{i}")
                  for i in range(NCC)]
            for ci in range(NCC):
                nc.sync.dma_start(xo[ci][:], xo_d[ci * P:(ci + 1) * P, :])

            # FFN1 first-half weights: allocate into the dead K slots now so
            # the DMAs run during the normalize/O-proj tail (per-slot WAR on
            # the last scores read; 512-col chunks land progressively)
            w1t0 = []
            for ci in range(NCC):
                wt = kx_pool.tile([P, 2048], bf16, tag=f"kx{ci}",
                                  name=f"w1t{ci}p0")
                for qc in range(4):
                    nc.sync.dma_start(
                        wt[:, qc * 512:(qc + 1) * 512],
                        w1_d[ci * P:(ci + 1) * P, qc * 512:(qc + 1) * 512])
                w1t0.append(wt)

            with tc.tile_pool(name="ops", bufs=4, space="PSUM") as o_ps:
                for co in range(NCC):
                    ps = o_ps.tile([P, TQ], f32, tag="ps", name="o_ps")
                    for hi in range(NCC):
                        nc.tensor.matmul(ps[:], ow[hi][:, co * P:(co + 1) * P],
                                         attn_sb[hi][:], start=(hi == 0),
                                         stop=(hi == NCC - 1))
                    nc.vector.scalar_tensor_tensor(x2[co][:], ps[:],
                                                   bcol(OB, co), xo[co][:],
                                                   OP.add, OP.add)

        # ------- LN2 over x2; fold into xbc = (x2 - mu2) * s2 (bf16)
        with tc.tile_pool(name="xq2", bufs=2) as xqp, \
             tc.tile_pool(name="ln2ps", bufs=1, space="PSUM") as ln2ps, \
             tc.tile_pool(name="ln2bc", bufs=1, space="PSUM") as ln2bc, \
             tc.tile_pool(name="ln2t", bufs=2) as ln2t:
            st2 = ln2ps.tile([33, TQ], f32, tag="st2", name="st2")
            xc2 = []
            for ci in range(NCC):
                xc = xqp.tile([P, TQ], bf16, tag=f"xc{ci}", name=f"xc2_{ci}")
                nc.scalar.copy(xc[:], x2[ci][:])
                xq = xqp.tile([P, TQ], bf16, tag="xq", name="xq2", bufs=2)
                nc.vector.tensor_mul(xq[:], xc[:], xc[:])
                nc.tensor.matmul(st2[0:1, :], ones_bf[:], xc[:],
                                 start=(ci == 0), stop=(ci == NCC - 1))
                nc.tensor.matmul(st2[32:33, :], ones_bf[:], xq[:],
                                 start=(ci == 0), stop=(ci == NCC - 1))
                xc2.append(xc)
            inv_n = 1.0 / D_MODEL
            mu2_sb = ln2t.tile([1, TQ], f32, tag="mu", name="mu2_sb")
            nc.vector.tensor_scalar_mul(mu2_sb[:], st2[0:1, :], inv_n)
            mu2sq = ln2t.tile([1, TQ], f32, tag="musq", name="mu2sq")
            nc.vector.tensor_mul(mu2sq[:], mu2_sb[:], mu2_sb[:])
            vpe = ln2t.tile([1, TQ], f32, tag="vpe", name="vpe2")
            nc.vector.tensor_scalar(vpe[:], st2[32:33, :], inv_n, 1e-5,
                                    OP.mult, OP.add)
            nc.vector.tensor_sub(vpe[:], vpe[:], mu2sq[:])
            rv = ln2t.tile([1, TQ], f32, tag="rv", name="rv2")
            nc.vector.reciprocal_approx_fast(rv[:], vpe[:])
            s2_f = ln2t.tile([1, TQ], f32, tag="ri", name="s2_f")
            nc.scalar.sqrt(s2_f[:], rv[:])
            s2_bf = ln2t.tile([1, TQ], bf16, tag="sbf", name="s2_bf")
            nmu2_bf = ln2t.tile([1, TQ], bf16, tag="nmb", name="nmu2_bf")
            with nc.allow_low_precision(reason="bf16 LN2 rows"):
                nc.vector.tensor_copy(s2_bf[:], s2_f[:])
                nc.vector.tensor_scalar_mul(nmu2_bf[:], mu2_sb[:], -1.0)
            nm2_bc = ln2bc.tile([P, TQ], f32, tag="nmbc", name="nm2_bc")
            nc.tensor.matmul(nm2_bc[:], ones_row[:], nmu2_bf[:])
            s2_bc = ln2bc.tile([P, TQ], f32, tag="sbc", name="s2_bc")
            nc.tensor.matmul(s2_bc[:], ones_row[:], s2_bf[:])
            with tc.tile_pool(name="xct2", bufs=2) as xct2_pool:
                for ci in range(NCC):
                    xc = xct2_pool.tile([P, TQ], bf16, tag="xc", name="xc2")
                    nc.vector.tensor_add(xc[:], xc2[ci][:], nm2_bc[:])
                    nc.vector.tensor_mul(xbc[ci][:], xc[:], s2_bc[:])

        # ---------------- FFN ----------------
        # h1 [4096, 512] lives in the K-arena slots as 8 groups of 4 f-chunks
        hg = [karena.tile([P, T], bf16, tag=f"k{i}", name=f"hg{i}")
              for i in range(NCC)]

        def h1sl(fch):
            return hg[fch // 4][:, (fch % 4) * 512:(fch % 4 + 1) * 512]

        with tc.tile_pool(name="h1ps", bufs=4, space="PSUM") as h1_ps:
            for fp in range(2):
                if fp == 0:
                    w1t = w1t0
                else:
                    w1t = []
                    for ci in range(NCC):
                        wt = kx_pool.tile([P, 2048], bf16, tag=f"kx{ci}",
                                          name=f"w1t{ci}p{fp}")
                        for qc in range(4):
                            nc.sync.dma_start(
                                wt[:, qc * 512:(qc + 1) * 512],
                                w1_d[ci * P:(ci + 1) * P,
                                     fp * 2048 + qc * 512:
                                     fp * 2048 + (qc + 1) * 512])
                        w1t.append(wt)
                for fo in range(16):
                    fch = fp * 16 + fo
                    ps = h1_ps.tile([P, TQ], f32, tag="ps", name="h1_ps")
                    for ci in range(NCC):
                        nc.tensor.matmul(ps[:],
                                         w1t[ci][:, fo * P:(fo + 1) * P],
                                         xbc[ci][:], start=(ci == 0),
                                         stop=(ci == NCC - 1))
                    nc.scalar.activation(h1sl(fch), ps[:], AF.Gelu,
                                         bias=bcol(B1, fch))

        with tc.tile_pool(name="outps", bufs=1, space="PSUM") as out_ps, \
             tc.tile_pool(name="outsb", bufs=1) as out_pool:
            ops = [out_ps.tile([P, TQ], f32, tag=f"o{co}", name=f"out_ps{co}")
                   for co in range(NCC)]
            for fch in range(NFC):
                wt = wqa_pool.tile([P, D_MODEL], bf16, tag=f"wq{fch % 8}",
                                   name=f"w2t{fch}")
                nc.sync.dma_start(wt[:], w2_d[fch * P:(fch + 1) * P, :])
                for co in range(NCC):
                    nc.tensor.matmul(ops[co][:], wt[:, co * P:(co + 1) * P],
                                     h1sl(fch),
                                     start=(fch == 0), stop=(fch == NFC - 1))
            for co in range(NCC):
                osb = out_pool.tile([P, TQ], f32, tag=f"os{co}",
                                    name=f"osb{co}")
                nc.vector.scalar_tensor_tensor(osb[:], ops[co][:],
                                               bcol(B2, co), x2[co][:],
                                               OP.add, OP.add)
                nc.sync.dma_start(out_d[co * P:(co + 1) * P, :], osb[:])


def _prep_inputs(x, qkv_w, qkv_b, o_w, o_b, ln1_g, ln1_b,
                 ffn_w1, ffn_b1, ffn_w2, ffn_b2, ln2_g, ln2_b):
    import ml_dtypes
    bf = ml_dtypes.bfloat16
    f8 = np.float64

    # fold LN gammas into the following projection weights, LN betas and
    # projection biases into per-output-feature constants (data-independent)
    Wg = qkv_w.astype(f8) * ln1_g.astype(f8)[None, :]
    cvec = qkv_w.astype(f8) @ ln1_b.astype(f8) + qkv_b.astype(f8)
    qkv_wT = np.ascontiguousarray(Wg.T.astype(np.float32)).astype(bf)
    wbar_k = np.ascontiguousarray(
        Wg[D_MODEL:2 * D_MODEL].sum(axis=1).astype(np.float32)[None, :]
    ).astype(bf)
    ob_eff = (o_b.astype(f8) + o_w.astype(f8) @ cvec[2 * D_MODEL:]
              ).astype(np.float32)

    W1g = ffn_w1.astype(f8) * ln2_g.astype(f8)[None, :]
    b1_eff = (ffn_w1.astype(f8) @ ln2_b.astype(f8)
              + ffn_b1.astype(f8)).astype(np.float32)
    w1T = np.ascontiguousarray(W1g.T.astype(np.float32)).astype(bf)

    o_wT = np.ascontiguousarray(o_w.T).astype(bf)
    w2T = np.ascontiguousarray(ffn_w2.T).astype(bf)

    def cols(v, n):
        return np.ascontiguousarray(v.reshape(n, P).T.astype(np.float32))

    biases = np.zeros((P, 64), np.float32)
    biases[:, QB:QB + 8] = cols(cvec[0:D_MODEL].astype(np.float32), 8)
    biases[:, OB:OB + 8] = cols(ob_eff, 8)
    biases[:, B1:B1 + 32] = cols(b1_eff, 32)
    biases[:, B2:B2 + 8] = cols(ffn_b2, 8)

    in_maps = []
    for c in range(N_CORES):
        b, s = c // GROUPS, c % GROUPS
        xs = np.ascontiguousarray(x[b][s * TQ:(s + 1) * TQ, :].T)
        in_maps.append({
            "x_fm": xs.astype(bf),
            "x_own": xs.astype(np.float32),
            "qkv_wT": qkv_wT,
            "o_wT": o_wT,
            "w1T": w1T,
            "w2T": w2T,
            "biases": biases,
            "wbar_k": wbar_k,
        })
    return in_maps


def kernel(**inputs):
    from concourse.bass_utils import run_bass_kernel_spmd

    if "nc" not in _cache:
        _cache["nc"] = _build()
    nc = _cache["nc"]

    inputs = {k: np.asarray(v, dtype=np.float32) for k, v in inputs.items()}
    in_maps = _prep_inputs(**inputs)

    res = run_bass_kernel_spmd(nc, in_maps, core_ids=list(range(N_CORES)),
                               **_cache.get("run_kwargs", {}))
    _cache["last_results"] = res

    out = np.empty((B, T, D_MODEL), np.float32)
    for c in range(N_CORES):
        b, s = c // GROUPS, c % GROUPS
        out[b, s * TQ:(s + 1) * TQ, :] = res.results[c]["out"].T
    return out
